# revision 22
# baseline (speedup 1.0000x reference)
"""Trainium2 Bass kernel for the trajectory-decoder LSTM problem.

Math (mirrors the reference, with algebraic folds):
  dec_inp feeds gates only through W_emb; W_sp/W_emb/W_hp collapse:
    W_es = W_emb @ W_sp            [4H, 2]
    gates_t = zx@W_zx.T + bias + r_{t-1}@W_es.T + h_{t-1}@W_hh.T
  For t>=1, r_{t-1} = h_{t-1}@W_hp.T + b_hp, so with
    W_hh' = W_hh + W_es @ W_hp,  bias1 = b_ih + b_hh + W_emb@b_sp + W_es@b_hp
  every step becomes uniform:  gates_t = zx@W_zx.T + bias1 + h_{t-1}@W_hh'.T
  The step-0 state needs no r_init projection: since the W_es@b_hp term
  of bias1 cancels,  gates_0 = zxp + bias1 + h_init@W_hh.T
                               + (lpr - b_hp)@W_es.T
  and the step-0 update adds h_0@W_hh'.T - h_init@W_hh.T - lprm@W_es.T,
  whose last two terms depend only on consts + h_init and issue right
  after sigmoid_0 (off the elementwise critical path).
  `last_pos` is dead code (output is just the stacked rel_pos).

Device strategy (pure data-parallel over 8 cores, 4096 batch each):
  - batch on the free dim, features on partitions
  - per 256-batch wave, the 4 gate pre-activations live RESIDENT in one
    2-bank PSUM tile; each step the PE accumulates (h_t - h_{t-1}) @ W_hh'.T
    into it (start=False), so no per-step zxp re-add on the vector engine.
    A K=2 "bank-open" matmul (bias x 0/1 indicator) clears has_written for
    the whole bank and injects the per-gate bias.
  - the g-gate weights are doubled on host so ONE sigmoid over all 4 banks
    yields sig(i),sig(f),sig(o),sig(2g); tanh(g)=2*sig(2g)-1 folds into the
    DVE scalar_tensor_tensor ops for m1 and c.
  - per-step elementwise: sigmoid (ACT), then m1/m2/c and h/dh as
    back-to-back DVE ops (same-queue chaining avoids cross-engine
    semaphore hops on the critical path); tanh on ACT.
  - rel_pos: h-stationary matmuls (batch on out partitions, N=2) into
    small scratch PSUM tiles, copied out via ACT+DVE, one contiguous
    partition-major DMA per wave.
  - waves are software-pipelined on a virtual timeline: every op gets
    an estimated execution time tau and ops are emitted sorted by tau,
    so each engine's in-order queue matches true readiness order.
    Chains step every L=5us; waves start every WS=23.4us (PE-
    feasibility bound). Filler matmuls (mlp/init) are chopped into
    <=0.7us chunks; weight DMAs go on the idle GpSimd queue, chunked
    so pair 0's first mlp unit starts after ~0.7MB.
  - all matmul operands fp16 (full PE rate), PSUM accumulation fp32;
    c kept in fp16, b_hp added on host after gather.
"""

import os
import numpy as np

B = 32768
NCORES = 8
BC = B // NCORES          # 4096 batch per core
WAVE = 256                # batch per recurrence chain (2 PSUM banks of gates)
NW = BC // WAVE           # 16 waves
PAIR = 2 * WAVE           # phase-A (mlp) runs at N=512 across wave pairs
T = 12                    # decode steps
H = 128
G4 = 4 * H                # 512 gate features
ZX = 1056
KP = 1152                 # ZX padded to 9*128
KT = KP // 128            # 9 contraction tiles
MLP = 1024
EMB = 64

_cache = {}


def _build_nc():
    import concourse.bass as bass
    import concourse.bacc as bacc
    import concourse.mybir as mybir
    import concourse.tile as tile
    from concourse.bass import ts

    f16 = mybir.dt.float16
    f32 = mybir.dt.float32
    AF = mybir.ActivationFunctionType
    OP = mybir.AluOpType

    nc = bacc.Bacc("TRN2", target_bir_lowering=False)

    zxT = nc.dram_tensor("zxT", [KP, BC], f16, kind="ExternalInput")
    lprT = nc.dram_tensor("lprT", [2, BC], f16, kind="ExternalInput")
    w1t = nc.dram_tensor("w1t", [128, KT, MLP], f16, kind="ExternalInput")
    wzxt = nc.dram_tensor("wzxt", [128, KT, G4], f16, kind="ExternalInput")
    w2t = nc.dram_tensor("w2t", [128, 8, H], f16, kind="ExternalInput")
    whht = nc.dram_tensor("whht", [128, G4], f16, kind="ExternalInput")
    whpt = nc.dram_tensor("whpt", [128, 2], f16, kind="ExternalInput")
    k3 = nc.dram_tensor("k3", [2, G4], f16, kind="ExternalInput")   # -W_es.T
    wes = nc.dram_tensor("wes", [2, G4], f16, kind="ExternalInput")  # +W_es.T
    bias2 = nc.dram_tensor("bias2", [2, 2 * 128], f16, kind="ExternalInput")
    ind = nc.dram_tensor("ind", [2, 2 * WAVE], f16, kind="ExternalInput")
    b1 = nc.dram_tensor("b1", [128, 8], f32, kind="ExternalInput")
    b2 = nc.dram_tensor("b2", [128, 1], f32, kind="ExternalInput")
    whh0 = nc.dram_tensor("whh0", [128, G4], f16, kind="ExternalInput")
    whhn = nc.dram_tensor("whhn", [128, G4], f16, kind="ExternalInput")
    # per wave: [partition=batch, (blk, t, j)] — partition-major so the
    # rel output DMA is one clean 192B-per-partition descriptor
    pred = nc.dram_tensor("pred", [NW, 128, 4 * T], f32, kind="ExternalOutput")

    with tile.TileContext(nc) as tc:
        with (
            tc.tile_pool(name="consts", bufs=1) as cpool,
            tc.tile_pool(name="zx", bufs=2) as zxpool,
            tc.tile_pool(name="h1", bufs=2) as h1pool,
            tc.tile_pool(name="hc", bufs=10) as hcpool,
            tc.tile_pool(name="acts", bufs=6) as apool,
            tc.tile_pool(name="outs", bufs=3) as opool,
            tc.tile_pool(name="scrps", bufs=2, space="PSUM") as scrpool,
            tc.tile_pool(name="gateps", bufs=3, space="PSUM") as gatepool,
        ):
            # ---- constants: small ones loaded upfront; the big mlp/zxp
            # weights are DMA'd in chunks as tau-events so pair 0's first
            # mlp unit starts after ~0.7MB instead of ~5MB ----
            w1t_c = [cpool.tile([128, KT, 256], f16, name="w1tc%d" % i)
                     for i in range(4)]
            wzxt_s = cpool.tile([128, KT, G4], f16)
            w2t_s = cpool.tile([128, 8, H], f16)
            whht_s = cpool.tile([128, G4], f16)
            nc.sync.dma_start(whht_s[:], whht[:])
            whpt_s = cpool.tile([128, 2], f16)
            nc.sync.dma_start(whpt_s[:], whpt[:])
            k3_s = cpool.tile([2, G4], f16)
            nc.sync.dma_start(k3_s[:], k3[:])
            wes_s = cpool.tile([2, G4], f16)
            nc.sync.dma_start(wes_s[:], wes[:])
            bias2_s = cpool.tile([2, 2 * 128], f16)
            nc.sync.dma_start(bias2_s[:], bias2[:])
            ind_s = cpool.tile([2, 2 * WAVE], f16)
            nc.sync.dma_start(ind_s[:], ind[:])
            b1_s = cpool.tile([128, 8], f32)
            nc.sync.dma_start(b1_s[:], b1[:])
            b2_s = cpool.tile([128, 1], f32)
            nc.sync.dma_start(b2_s[:], b2[:])
            whh0_s = cpool.tile([128, G4], f16)
            nc.sync.dma_start(whh0_s[:], whh0[:])
            whhn_s = cpool.tile([128, G4], f16)
            nc.sync.dma_start(whhn_s[:], whhn[:])
            lpr_s = cpool.tile([2, BC], f16)
            nc.sync.dma_start(lpr_s[:], lprT[:])

            zxT_v = zxT.rearrange("(k p) b -> p k b", p=128)

            # Virtual-timeline schedule: every op gets an estimated
            # execution time tau (ns) and ops are emitted sorted by tau,
            # so each engine's in-order queue matches true readiness
            # order. Chains step every L; waves start every WS (set by
            # PE feasibility: ~22us of PE work per wave). Filler matmul
            # bursts (mlp/init/rel) are chopped into <=0.7us chunks so a
            # chain's gate matmuls are never queued behind a long burst.
            NP = NW // 2
            state = [dict() for _ in range(NW)]
            pair_state = [dict() for _ in range(NP)]
            events = []

            L = 5000
            WS = 23400

            def ev(tau, fn):
                events.append((tau, len(events), fn))

            def mk_w1t_dma(jh, eng):
                def fn():
                    eng.dma_start(
                        w1t_c[jh][:], w1t[:, :, ts(jh, 256)]
                    )
                return fn

            def mk_wzxt_dma():
                def fn():
                    nc.scalar.dma_start(wzxt_s[:], wzxt[:])
                return fn

            def mk_w2t_dma():
                def fn():
                    nc.sync.dma_start(w2t_s[:], w2t[:])
                return fn

            def mk_zxw(p):
                def fn():
                    st = pair_state[p]
                    za = zxpool.tile([128, 5, PAIR], f16, tag="zxwa", name="zxwa")
                    zb = zxpool.tile([128, 4, PAIR], f16, tag="zxwb", name="zxwb")
                    engb = nc.scalar if p == 0 else nc.gpsimd
                    nc.gpsimd.dma_start(za[:], zxT_v[:, 0:5, ts(p, PAIR)])
                    engb.dma_start(zb[:], zxT_v[:, 5:KT, ts(p, PAIR)])
                    st["zxwa"], st["zxwb"] = za, zb
                    st["h1"] = h1pool.tile([128, 8, PAIR], f16, tag="h1", name="h1")
                return fn

            def zxk(st, k):
                return (st["zxwa"], k) if k < 5 else (st["zxwb"], k - 5)

            def mk_mlp1_mm(p, j, k0):
                def fn():
                    st = pair_state[p]
                    if k0 == 0:
                        st["ps"] = scrpool.tile([128, PAIR], f32, tag="scratch", name="ps")
                    for k in range(k0, min(k0 + 3, KT)):
                        zt, zk = zxk(st, k)
                        nc.tensor.matmul(
                            st["ps"][:],
                            w1t_c[j // 2][:, k, ts(j % 2, 128)], zt[:, zk, :],
                            start=(k == 0), stop=(k == KT - 1),
                        )
                return fn

            def mk_mlp1_ts(p, j):
                def fn():
                    st = pair_state[p]
                    if j % 2 == 0:
                        nc.scalar.activation(
                            st["h1"][:, j, :], st["ps"][:], AF.Relu,
                            bias=b1_s[:, j : j + 1],
                        )
                    else:
                        nc.vector.tensor_scalar(
                            st["h1"][:, j, :], st["ps"][:], b1_s[:, j : j + 1],
                            0.0, OP.add, OP.max,
                        )
                return fn

            def mk_mlp2_mm(p, j0, nj):
                def fn():
                    st = pair_state[p]
                    if j0 == 0:
                        st["ps"] = scrpool.tile([128, PAIR], f32, tag="scratch", name="ps")
                    for j in range(j0, j0 + nj):
                        nc.tensor.matmul(
                            st["ps"][:], w2t_s[:, j, :], st["h1"][:, j, :],
                            start=(j == 0), stop=(j == 7),
                        )
                return fn

            def mk_mlp2_ts(p):
                def fn():
                    st = pair_state[p]
                    hi = h1pool.tile([128, PAIR], f16, tag="hinit", name="hinit")
                    nc.vector.tensor_scalar(
                        hi[:], st["ps"][:], b2_s[:, 0:1], 0.0, OP.add, OP.max
                    )
                    st["h_init"] = hi
                return fn

            def mk_bias(w):
                def fn():
                    st = state[w]
                    gates = gatepool.tile([128, 4 * WAVE], f32, tag="gates", name="gates")
                    st["gates"] = gates
                    for bk in range(2):
                        nc.tensor.matmul(
                            gates[:, ts(bk, 2 * WAVE)], bias2_s[:, ts(bk, 128)],
                            ind_s[:], start=True, stop=False, skip_group_check=True,
                        )
                return fn

            def mk_zxp(w, g):
                def fn():
                    st = state[w]
                    pst = pair_state[w // 2]
                    hs = ts(w % 2, WAVE)
                    gp = st["gates"][:, ts(g, WAVE)]
                    for k in range(KT):
                        zt, zk = zxk(pst, k)
                        nc.tensor.matmul(
                            gp[:], wzxt_s[:, k, ts(g, 128)], zt[:, zk, hs],
                            start=False, stop=False, skip_group_check=True,
                        )
                return fn

            # gates_0 = zxp + bias + h_init@W_hh.T + (lpr-b_hp)@W_es.T
            # (the W_es@b_hp term of bias1 cancels; no r_init projection
            # needed, so the pair handoff is just mlp2 -> init_hh -> sig)
            def mk_init_wes(w):
                def fn():
                    st = state[w]
                    for g in range(4):
                        nc.tensor.matmul(
                            st["gates"][:, ts(g, WAVE)], wes_s[:, ts(g, 128)],
                            lpr_s[:, ts(w, WAVE)],
                            start=False, stop=False, skip_group_check=True,
                        )
                return fn

            def mk_init_hh(w):
                def fn():
                    st = state[w]
                    st["h_prev"] = pair_state[w // 2]["h_init"][:, ts(w % 2, WAVE)]
                    for g in range(4):
                        nc.tensor.matmul(
                            st["gates"][:, ts(g, WAVE)], whh0_s[:, ts(g, 128)],
                            st["h_prev"][:],
                            start=False, stop=False, skip_group_check=True,
                        )
                return fn

            # step-0 removal: gates += -h_init@W_hh.T - (lpr-b_hp)@W_es.T;
            # depends only on consts + h_init, issues right after sigmoid_0
            def mk_t0rm(w):
                def fn():
                    st = state[w]
                    for g in range(4):
                        gp = st["gates"][:, ts(g, WAVE)]
                        nc.tensor.matmul(
                            gp[:], whhn_s[:, ts(g, 128)], st["h_prev"][:],
                            start=False, stop=False, skip_group_check=True,
                        )
                        nc.tensor.matmul(
                            gp[:], k3_s[:, ts(g, 128)], lpr_s[:, ts(w, WAVE)],
                            start=False, stop=False, skip_group_check=True,
                        )
                return fn

            def mk_sig(w, t):
                def fn():
                    st = state[w]
                    sig = apool.tile([128, 4 * WAVE], f16, tag="sig", name="sig")
                    nc.scalar.activation(sig[:], st["gates"][:], AF.Sigmoid)
                    st["sig"] = sig
                return fn

            def mk_m1(w, t):
                def fn():
                    st = state[w]
                    m1 = apool.tile([128, WAVE], f16, tag="m1", name="m1")
                    nc.vector.scalar_tensor_tensor(
                        m1[:], st["sig"][:, 3 * WAVE : 4 * WAVE], 0.5,
                        st["sig"][:, 0:WAVE], OP.subtract, OP.mult,
                    )
                    st["m1"] = m1
                return fn

            def mk_m2(w, t):
                # on DVE right after m1: same-queue back-to-back beats the
                # two cross-engine hops of a Pool m2 on the c critical path
                def fn():
                    st = state[w]
                    m2 = apool.tile([128, WAVE], f16, tag="m2", name="m2")
                    nc.vector.tensor_tensor(
                        m2[:], st["sig"][:, WAVE : 2 * WAVE], st["c_prev"][:], OP.mult
                    )
                    st["m2"] = m2
                return fn

            def mk_c(w, t):
                def fn():
                    st = state[w]
                    c_new = hcpool.tile([128, WAVE], f16, tag="c", name="c")
                    if t == 0:
                        nc.vector.tensor_scalar_mul(c_new[:], st["m1"][:], 2.0)
                        st["h_all"] = opool.tile(
                            [128, T * WAVE], f16, tag="hall", name="hall"
                        )
                    else:
                        nc.vector.scalar_tensor_tensor(
                            c_new[:], st["m1"][:], 2.0, st["m2"][:], OP.mult, OP.add
                        )
                    st["c_prev"] = c_new
                return fn

            def mk_tanh(w, t):
                def fn():
                    st = state[w]
                    tanhc = apool.tile([128, WAVE], f16, tag="tanhc", name="tanhc")
                    nc.scalar.activation(tanhc[:], st["c_prev"][:], AF.Tanh)
                    st["tanhc"] = tanhc
                return fn

            def mk_h(w, t):
                def fn():
                    st = state[w]
                    h_new = st["h_all"][:, ts(t, WAVE)]
                    nc.vector.tensor_tensor(
                        h_new[:], st["sig"][:, 2 * WAVE : 3 * WAVE], st["tanhc"][:],
                        OP.mult,
                    )
                    st["h_new"] = h_new
                return fn

            def mk_dh(w, t):
                # DVE back-to-back after h: GpSimd's ~800ns TT plus two
                # cross-engine hops was the fattest slice of step latency
                def fn():
                    st = state[w]
                    dh = apool.tile([128, WAVE], f16, tag="dh", name="dh")
                    nc.vector.tensor_tensor(
                        dh[:], st["h_new"][:], st["h_prev"][:], OP.subtract
                    )
                    st["dh"] = dh
                    st["h_prev"] = st["h_new"]
                return fn

            def mk_mm(w, t):
                def fn():
                    st = state[w]
                    mv = st["h_new"] if t == 0 else st["dh"]
                    for g in range(4):
                        nc.tensor.matmul(
                            st["gates"][:, ts(g, WAVE)], whht_s[:, ts(g, 128)],
                            mv[:],
                            start=False, stop=(t == T - 2), skip_group_check=True,
                        )
                    if t == 0:
                        st["h_prev"] = st["h_new"]
                return fn

            # rel_pos: h-stationary matmuls (batch on out partitions,
            # N=2) into two small scratch PSUM tiles so the gates tile
            # frees at its last sigmoid; split copy across ACT/DVE.
            def mk_rel_mm(w, e0):
                def fn():
                    st = state[w]
                    blk = e0 // T
                    if e0 % T == 0:
                        st["relp%d" % blk] = scrpool.tile(
                            [128, 2 * T], f32, tag="scratch", name="relp"
                        )
                    rp = st["relp%d" % blk]
                    for e in range(e0, e0 + 6):
                        t = e % T
                        nc.tensor.matmul(
                            rp[:, 2 * t : 2 * t + 2],
                            st["h_all"][:, t * WAVE + blk * 128 :
                                         t * WAVE + blk * 128 + 128],
                            whpt_s[:], start=True, stop=True, skip_group_check=True,
                        )
                return fn

            def mk_rel_copy(w):
                # GPSIMD can't read PSUM; split the copy across ACT and DVE
                def fn():
                    st = state[w]
                    sb = opool.tile([128, 4 * T], f32, tag="predsb", name="predsb")
                    nc.scalar.copy(sb[:, 0 : 2 * T], st["relp0"][:])
                    nc.vector.tensor_copy(sb[:, 2 * T : 4 * T], st["relp1"][:])
                    st["predsb"] = sb
                return fn

            def mk_rel_dma(w):
                def fn():
                    st = state[w]
                    nc.sync.dma_start(pred[w], st["predsb"][:])
                return fn

            for w in range(NW):
                S = w * WS
                ev(S - 8000, mk_bias(w))
                for g in range(4):
                    ev(S - 7800 + 900 * g, mk_zxp(w, g))
                ev(S - 3600, mk_init_wes(w))
                ev(S - 2600, mk_init_hh(w))
                for t in range(T):
                    tau = S + t * L
                    ev(tau, mk_sig(w, t))
                    if t == 0:
                        ev(tau + 2700, mk_t0rm(w))
                    ev(tau + 1250, mk_m1(w, t))
                    if t > 0:
                        ev(tau + 1700, mk_m2(w, t))
                    ev(tau + 2150, mk_c(w, t))
                    ev(tau + 2550, mk_tanh(w, t))
                    ev(tau + 3100, mk_h(w, t))
                    if t == 0:
                        ev(tau + 3450, mk_mm(w, t))
                    elif t < T - 1:
                        ev(tau + 3450, mk_dh(w, t))
                        ev(tau + 3950, mk_mm(w, t))
                R = S + 11 * L
                for c in range(4):
                    ev(R + 3300 + 600 * c, mk_rel_mm(w, 6 * c))
                ev(R + 5700, mk_rel_copy(w))
                ev(R + 6100, mk_rel_dma(w))
            for p in range(NP):
                E = 2 * p * WS
                ev(-42000 if p == 0 else E - 27800, mk_zxw(p))
                for j in range(8):
                    u = E - 27000 + 2400 * j
                    for ci, k0 in enumerate((0, 3, 6)):
                        ev(u + 750 * ci, mk_mlp1_mm(p, j, k0))
                    ev(u + 2100, mk_mlp1_ts(p, j))
                for ci, (j0, nj) in enumerate(((0, 3), (3, 3), (6, 2))):
                    ev(E - 7600 + 700 * ci, mk_mlp2_mm(p, j0, nj))
                ev(E - 5800, mk_mlp2_ts(p))
            ev(-41999, mk_w1t_dma(0, nc.sync))
            ev(-41998, mk_wzxt_dma())
            for jh in range(1, 4):
                ev(-41000 + 300 * jh, mk_w1t_dma(jh, nc.gpsimd))
            ev(-38800, mk_w2t_dma())

            for _, _, fn in sorted(events, key=lambda e: (e[0], e[1])):
                fn()

    nc.compile()
    return nc


def _prep(inputs):
    """Host-side weight folding + layout prep. Returns per-core input maps."""
    f = np.float64
    W_ih = np.asarray(inputs["W_ih"], f)
    W_hh = np.asarray(inputs["W_hh"], f)
    b_ih = np.asarray(inputs["b_ih"], f)
    b_hh = np.asarray(inputs["b_hh"], f)
    W1 = np.asarray(inputs["W1"], f)
    b1 = np.asarray(inputs["b1"], f)
    W2 = np.asarray(inputs["W2"], f)
    b2 = np.asarray(inputs["b2"], f)
    W_sp = np.asarray(inputs["W_sp"], f)
    b_sp = np.asarray(inputs["b_sp"], f)
    W_hp = np.asarray(inputs["W_hp"], f)
    b_hp = np.asarray(inputs["b_hp"], f)

    W_zx = W_ih[:, :ZX]
    W_emb = W_ih[:, ZX:]
    W_es = W_emb @ W_sp                       # [4H, 2]
    W_hh_f = W_hh + W_es @ W_hp               # [4H, H]
    bias1 = b_ih + b_hh + W_emb @ b_sp + W_es @ b_hp

    # reorder pytorch gates (i, f, g, o) -> (i, f, o, g)
    perm = np.r_[0:H, H : 2 * H, 3 * H : 4 * H, 2 * H : 3 * H]
    W_zx = W_zx[perm]
    W_hh_f = W_hh_f[perm]
    W_es = W_es[perm]
    bias1 = bias1[perm]
    # double the g-gate block: its bank then holds 2*g_pre, so
    # tanh(g) = 2*sigmoid(2*g_pre) - 1 comes out of the one big sigmoid
    dbl = np.ones((G4, 1))
    dbl[3 * H :] = 2.0
    W_zx = W_zx * dbl
    W_hh_f = W_hh_f * dbl
    W_es = W_es * dbl
    bias1 = bias1 * dbl[:, 0]

    def kxm(Wt, kp):  # [K, M] -> [128, K/128, M] fp16, zero-padded to kp rows
        K, M = Wt.shape
        out = np.zeros((kp, M), f)
        out[:K] = Wt
        return np.ascontiguousarray(
            out.reshape(kp // 128, 128, M).transpose(1, 0, 2)
        ).astype(np.float16)

    W_hh_pd = W_hh[perm] * dbl
    consts = {
        "whh0": np.ascontiguousarray(W_hh_pd.T).astype(np.float16),
        "whhn": np.ascontiguousarray(-W_hh_pd.T).astype(np.float16),
        "w1t": kxm(W1.T, KP),
        "wzxt": kxm(W_zx.T, KP),
        "w2t": kxm(W2.T, MLP),
        "whht": np.ascontiguousarray(W_hh_f.T).astype(np.float16),
        "whpt": np.ascontiguousarray(W_hp.T).astype(np.float16),
        "k3": np.ascontiguousarray(-W_es.T).astype(np.float16),
        "wes": np.ascontiguousarray(W_es.T).astype(np.float16),
        # bank-open bias: bias2[r, bk*128+m] = bias1[(2*bk + r)*128 + m]
        "bias2": np.ascontiguousarray(
            bias1.reshape(4, 128).reshape(2, 2, 128).transpose(1, 0, 2).reshape(2, 256)
        ).astype(np.float16),
        # 0/1 indicator selecting which half-bank gets which bias row
        "ind": np.kron(np.eye(2), np.ones((1, WAVE))).astype(np.float16),
        "b1": np.ascontiguousarray(b1.reshape(8, 128).T).astype(np.float32),
        "b2": b2.reshape(128, 1).astype(np.float32),
    }

    enc = np.asarray(inputs["enc_h_feat"], np.float32)
    z = np.asarray(inputs["z"], np.float32)
    lpr = np.asarray(inputs["last_pos_rel"], np.float32)
    zxT = np.zeros((KP, B), np.float16)
    zxT[:MLP] = enc.T
    zxT[MLP:ZX] = z.T
    lprT = np.ascontiguousarray((lpr - b_hp[None, :]).T).astype(np.float16)

    in_maps = []
    for c in range(NCORES):
        s = slice(c * BC, (c + 1) * BC)
        m = dict(consts)
        m["zxT"] = np.ascontiguousarray(zxT[:, s])
        m["lprT"] = np.ascontiguousarray(lprT[:, s])
        in_maps.append(m)
    return in_maps


def run(inputs, trace=False):
    from concourse.bass_utils import run_bass_kernel_spmd

    if "nc" not in _cache:
        _cache["nc"] = _build_nc()
    in_maps = _prep(inputs)
    res = run_bass_kernel_spmd(
        _cache["nc"], in_maps, core_ids=list(range(NCORES)), trace=trace
    )
    # per core: [NW, 128, (blk, t, j)]; batch = w*256 + blk*128 + p
    def decode(a):
        a = a.reshape(NW, 128, 2, T, 2)              # w p blk t j
        return a.transpose(0, 2, 1, 3, 4).reshape(BC, T, 2)
    pred = np.concatenate(
        [decode(r["pred"]) for r in res.results], axis=0
    )  # [B, T, 2]
    out = pred.transpose(1, 0, 2) + np.asarray(inputs["b_hp"], np.float32)[None, None, :]
    return np.ascontiguousarray(out), res


def kernel(**inputs) -> np.ndarray:
    out, _ = run(inputs, trace=False)
    return out



# revision 23
# speedup vs baseline: 1.0102x; 1.0102x over previous
"""Trainium2 Bass kernel for the trajectory-decoder LSTM problem.

Math (mirrors the reference, with algebraic folds):
  dec_inp feeds gates only through W_emb; W_sp/W_emb/W_hp collapse:
    W_es = W_emb @ W_sp            [4H, 2]
    gates_t = zx@W_zx.T + bias + r_{t-1}@W_es.T + h_{t-1}@W_hh.T
  For t>=1, r_{t-1} = h_{t-1}@W_hp.T + b_hp, so with
    W_hh' = W_hh + W_es @ W_hp,  bias1 = b_ih + b_hh + W_emb@b_sp + W_es@b_hp
  every step becomes uniform:  gates_t = zx@W_zx.T + bias1 + h_{t-1}@W_hh'.T
  The step-0 state needs no r_init projection: since the W_es@b_hp term
  of bias1 cancels,  gates_0 = zxp + bias1 + h_init@W_hh.T
                               + (lpr - b_hp)@W_es.T
  and the step-0 update adds h_0@W_hh'.T - h_init@W_hh.T - lprm@W_es.T,
  whose last two terms depend only on consts + h_init and issue right
  after sigmoid_0 (off the elementwise critical path).
  `last_pos` is dead code (output is just the stacked rel_pos).

Device strategy (pure data-parallel over 8 cores, 4096 batch each):
  - batch on the free dim, features on partitions
  - per 256-batch wave, the 4 gate pre-activations live RESIDENT in one
    2-bank PSUM tile; each step the PE accumulates (h_t - h_{t-1}) @ W_hh'.T
    into it (start=False), so no per-step zxp re-add on the vector engine.
    A K=2 "bank-open" matmul (bias x 0/1 indicator) clears has_written for
    the whole bank and injects the per-gate bias.
  - the g-gate weights are doubled on host so ONE sigmoid over all 4 banks
    yields sig(i),sig(f),sig(o),sig(2g); tanh(g)=2*sig(2g)-1 folds into the
    DVE scalar_tensor_tensor ops for m1 and c.
  - per-step elementwise: sigmoid (ACT), then m1/m2/c and h/dh as
    back-to-back DVE ops (same-queue chaining avoids cross-engine
    semaphore hops on the critical path); tanh on ACT.
  - rel_pos: h-stationary matmuls (batch on out partitions, N=2) into
    small scratch PSUM tiles, copied out via ACT+DVE, one contiguous
    partition-major DMA per wave.
  - waves are software-pipelined on a virtual timeline: every op gets
    an estimated execution time tau and ops are emitted sorted by tau,
    so each engine's in-order queue matches true readiness order.
    Chains step every L=5us; waves start every WS=23.4us (PE-
    feasibility bound). Filler matmuls (mlp/init) are chopped into
    <=0.7us chunks; weight DMAs go on the idle GpSimd queue, chunked
    so pair 0's first mlp unit starts after ~0.7MB.
  - all matmul operands fp16 (full PE rate), PSUM accumulation fp32;
    c kept in fp16, b_hp added on host after gather.
"""

import os
import numpy as np

B = 32768
NCORES = 8
BC = B // NCORES          # 4096 batch per core
WAVE = 256                # batch per recurrence chain (2 PSUM banks of gates)
NW = BC // WAVE           # 16 waves
PAIR = 2 * WAVE           # phase-A (mlp) runs at N=512 across wave pairs
T = 12                    # decode steps
H = 128
G4 = 4 * H                # 512 gate features
ZX = 1056
KP = 1152                 # ZX padded to 9*128
KT = KP // 128            # 9 contraction tiles
MLP = 1024
EMB = 64

_cache = {}


def _build_nc():
    import concourse.bass as bass
    import concourse.bacc as bacc
    import concourse.mybir as mybir
    import concourse.tile as tile
    from concourse.bass import ts

    f16 = mybir.dt.float16
    f32 = mybir.dt.float32
    AF = mybir.ActivationFunctionType
    OP = mybir.AluOpType

    nc = bacc.Bacc("TRN2", target_bir_lowering=False)

    zxT = nc.dram_tensor("zxT", [KP, BC], f16, kind="ExternalInput")
    lprT = nc.dram_tensor("lprT", [2, BC], f16, kind="ExternalInput")
    w1t = nc.dram_tensor("w1t", [128, KT, MLP], f16, kind="ExternalInput")
    wzxt = nc.dram_tensor("wzxt", [128, KT, G4], f16, kind="ExternalInput")
    w2t = nc.dram_tensor("w2t", [128, 8, H], f16, kind="ExternalInput")
    whht = nc.dram_tensor("whht", [128, G4], f16, kind="ExternalInput")
    whpt = nc.dram_tensor("whpt", [128, 2], f16, kind="ExternalInput")
    k3 = nc.dram_tensor("k3", [2, G4], f16, kind="ExternalInput")   # -W_es.T
    wes = nc.dram_tensor("wes", [2, G4], f16, kind="ExternalInput")  # +W_es.T
    bias2 = nc.dram_tensor("bias2", [2, 2 * 128], f16, kind="ExternalInput")
    ind = nc.dram_tensor("ind", [2, 2 * WAVE], f16, kind="ExternalInput")
    b1 = nc.dram_tensor("b1", [128, 8], f32, kind="ExternalInput")
    b2 = nc.dram_tensor("b2", [128, 1], f32, kind="ExternalInput")
    whh0 = nc.dram_tensor("whh0", [128, G4], f16, kind="ExternalInput")
    whhn = nc.dram_tensor("whhn", [128, G4], f16, kind="ExternalInput")
    # per wave: [partition=batch, (blk, t, j)] — partition-major so the
    # rel output DMA is one clean 192B-per-partition descriptor
    pred = nc.dram_tensor("pred", [NW, 128, 4 * T], f32, kind="ExternalOutput")

    with tile.TileContext(nc) as tc:
        with (
            tc.tile_pool(name="consts", bufs=1) as cpool,
            tc.tile_pool(name="zx", bufs=2) as zxpool,
            tc.tile_pool(name="h1", bufs=2) as h1pool,
            tc.tile_pool(name="hc", bufs=10) as hcpool,
            tc.tile_pool(name="acts", bufs=6) as apool,
            tc.tile_pool(name="outs", bufs=3) as opool,
            tc.tile_pool(name="scrps", bufs=2, space="PSUM") as scrpool,
            tc.tile_pool(name="gateps", bufs=3, space="PSUM") as gatepool,
        ):
            # ---- constants: small ones loaded upfront; the big mlp/zxp
            # weights are DMA'd in chunks as tau-events so pair 0's first
            # mlp unit starts after ~0.7MB instead of ~5MB ----
            w1t_s = cpool.tile([128, KT, MLP], f16)
            wzxt_s = cpool.tile([128, KT, G4], f16)
            w2t_s = cpool.tile([128, 8, H], f16)
            whht_s = cpool.tile([128, G4], f16)
            nc.sync.dma_start(whht_s[:], whht[:])
            whpt_s = cpool.tile([128, 2], f16)
            nc.sync.dma_start(whpt_s[:], whpt[:])
            k3_s = cpool.tile([2, G4], f16)
            nc.sync.dma_start(k3_s[:], k3[:])
            wes_s = cpool.tile([2, G4], f16)
            nc.sync.dma_start(wes_s[:], wes[:])
            bias2_s = cpool.tile([2, 2 * 128], f16)
            nc.sync.dma_start(bias2_s[:], bias2[:])
            ind_s = cpool.tile([2, 2 * WAVE], f16)
            nc.sync.dma_start(ind_s[:], ind[:])
            b1_s = cpool.tile([128, 8], f32)
            nc.sync.dma_start(b1_s[:], b1[:])
            b2_s = cpool.tile([128, 1], f32)
            nc.sync.dma_start(b2_s[:], b2[:])
            whh0_s = cpool.tile([128, G4], f16)
            nc.sync.dma_start(whh0_s[:], whh0[:])
            whhn_s = cpool.tile([128, G4], f16)
            nc.sync.dma_start(whhn_s[:], whhn[:])
            lpr_s = cpool.tile([2, BC], f16)
            nc.sync.dma_start(lpr_s[:], lprT[:])

            zxT_v = zxT.rearrange("(k p) b -> p k b", p=128)

            # Virtual-timeline schedule: every op gets an estimated
            # execution time tau (ns) and ops are emitted sorted by tau,
            # so each engine's in-order queue matches true readiness
            # order. Chains step every L; waves start every WS (set by
            # PE feasibility: ~22us of PE work per wave). Filler matmul
            # bursts (mlp/init/rel) are chopped into <=0.7us chunks so a
            # chain's gate matmuls are never queued behind a long burst.
            NP = NW // 2
            state = [dict() for _ in range(NW)]
            pair_state = [dict() for _ in range(NP)]
            events = []

            L = 5000
            WS = 24200

            def ev(tau, fn):
                events.append((tau, len(events), fn))

            def mk_w1t_dma(jh):
                def fn():
                    eng = nc.gpsimd
                    eng.dma_start(
                        w1t_s[:, :, ts(jh, 256)], w1t[:, :, ts(jh, 256)]
                    )
                return fn

            def mk_wzxt_dma(half):
                def fn():
                    nc.gpsimd.dma_start(
                        wzxt_s[:, :, ts(half, 256)], wzxt[:, :, ts(half, 256)]
                    )
                return fn

            def mk_w2t_dma():
                def fn():
                    nc.sync.dma_start(w2t_s[:], w2t[:])
                return fn

            def mk_zxw(p):
                def fn():
                    st = pair_state[p]
                    zxw = zxpool.tile([128, KT, PAIR], f16, tag="zxw", name="zxw")
                    nc.gpsimd.dma_start(zxw[:], zxT_v[:, :, ts(p, PAIR)])
                    st["zxw"] = zxw
                    st["h1"] = h1pool.tile([128, 8, PAIR], f16, tag="h1", name="h1")
                return fn

            def mk_mlp1_mm(p, j, k0):
                def fn():
                    st = pair_state[p]
                    if k0 == 0:
                        st["ps"] = scrpool.tile([128, PAIR], f32, tag="scratch", name="ps")
                    for k in range(k0, min(k0 + 3, KT)):
                        nc.tensor.matmul(
                            st["ps"][:], w1t_s[:, k, ts(j, 128)], st["zxw"][:, k, :],
                            start=(k == 0), stop=(k == KT - 1),
                        )
                return fn

            def mk_mlp1_ts(p, j):
                def fn():
                    st = pair_state[p]
                    if j % 2 == 0:
                        nc.scalar.activation(
                            st["h1"][:, j, :], st["ps"][:], AF.Relu,
                            bias=b1_s[:, j : j + 1],
                        )
                    else:
                        nc.vector.tensor_scalar(
                            st["h1"][:, j, :], st["ps"][:], b1_s[:, j : j + 1],
                            0.0, OP.add, OP.max,
                        )
                return fn

            def mk_mlp2_mm(p, j0, nj):
                def fn():
                    st = pair_state[p]
                    if j0 == 0:
                        st["ps"] = scrpool.tile([128, PAIR], f32, tag="scratch", name="ps")
                    for j in range(j0, j0 + nj):
                        nc.tensor.matmul(
                            st["ps"][:], w2t_s[:, j, :], st["h1"][:, j, :],
                            start=(j == 0), stop=(j == 7),
                        )
                return fn

            def mk_mlp2_ts(p):
                def fn():
                    st = pair_state[p]
                    hi = h1pool.tile([128, PAIR], f16, tag="hinit", name="hinit")
                    nc.vector.tensor_scalar(
                        hi[:], st["ps"][:], b2_s[:, 0:1], 0.0, OP.add, OP.max
                    )
                    st["h_init"] = hi
                return fn

            def mk_bias(w):
                def fn():
                    st = state[w]
                    gates = gatepool.tile([128, 4 * WAVE], f32, tag="gates", name="gates")
                    st["gates"] = gates
                    for bk in range(2):
                        nc.tensor.matmul(
                            gates[:, ts(bk, 2 * WAVE)], bias2_s[:, ts(bk, 128)],
                            ind_s[:], start=True, stop=False, skip_group_check=True,
                        )
                return fn

            def mk_zxp(w, g):
                def fn():
                    st = state[w]
                    pst = pair_state[w // 2]
                    hs = ts(w % 2, WAVE)
                    gp = st["gates"][:, ts(g, WAVE)]
                    for k in range(KT):
                        nc.tensor.matmul(
                            gp[:], wzxt_s[:, k, ts(g, 128)], pst["zxw"][:, k, hs],
                            start=False, stop=False, skip_group_check=True,
                        )
                return fn

            # gates_0 = zxp + bias + h_init@W_hh.T + (lpr-b_hp)@W_es.T
            # (the W_es@b_hp term of bias1 cancels; no r_init projection
            # needed, so the pair handoff is just mlp2 -> init_hh -> sig)
            def mk_init_wes(w):
                def fn():
                    st = state[w]
                    for g in range(4):
                        nc.tensor.matmul(
                            st["gates"][:, ts(g, WAVE)], wes_s[:, ts(g, 128)],
                            lpr_s[:, ts(w, WAVE)],
                            start=False, stop=False, skip_group_check=True,
                        )
                return fn

            def mk_init_hh(w):
                def fn():
                    st = state[w]
                    st["h_prev"] = pair_state[w // 2]["h_init"][:, ts(w % 2, WAVE)]
                    for g in range(4):
                        nc.tensor.matmul(
                            st["gates"][:, ts(g, WAVE)], whh0_s[:, ts(g, 128)],
                            st["h_prev"][:],
                            start=False, stop=False, skip_group_check=True,
                        )
                return fn

            # step-0 removal: gates += -h_init@W_hh.T - (lpr-b_hp)@W_es.T;
            # depends only on consts + h_init, issues right after sigmoid_0
            def mk_t0rm(w):
                def fn():
                    st = state[w]
                    for g in range(4):
                        gp = st["gates"][:, ts(g, WAVE)]
                        nc.tensor.matmul(
                            gp[:], whhn_s[:, ts(g, 128)], st["h_prev"][:],
                            start=False, stop=False, skip_group_check=True,
                        )
                        nc.tensor.matmul(
                            gp[:], k3_s[:, ts(g, 128)], lpr_s[:, ts(w, WAVE)],
                            start=False, stop=False, skip_group_check=True,
                        )
                return fn

            def mk_sig(w, t):
                def fn():
                    st = state[w]
                    sig = apool.tile([128, 4 * WAVE], f16, tag="sig", name="sig")
                    nc.scalar.activation(sig[:], st["gates"][:], AF.Sigmoid)
                    st["sig"] = sig
                return fn

            def mk_m1(w, t):
                def fn():
                    st = state[w]
                    m1 = apool.tile([128, WAVE], f16, tag="m1", name="m1")
                    nc.vector.scalar_tensor_tensor(
                        m1[:], st["sig"][:, 3 * WAVE : 4 * WAVE], 0.5,
                        st["sig"][:, 0:WAVE], OP.subtract, OP.mult,
                    )
                    st["m1"] = m1
                return fn

            def mk_m2(w, t):
                # on DVE right after m1: same-queue back-to-back beats the
                # two cross-engine hops of a Pool m2 on the c critical path
                def fn():
                    st = state[w]
                    m2 = apool.tile([128, WAVE], f16, tag="m2", name="m2")
                    nc.vector.tensor_tensor(
                        m2[:], st["sig"][:, WAVE : 2 * WAVE], st["c_prev"][:], OP.mult
                    )
                    st["m2"] = m2
                return fn

            def mk_c(w, t):
                def fn():
                    st = state[w]
                    c_new = hcpool.tile([128, WAVE], f16, tag="c", name="c")
                    if t == 0:
                        nc.vector.tensor_scalar_mul(c_new[:], st["m1"][:], 2.0)
                        st["h_all"] = opool.tile(
                            [128, T * WAVE], f16, tag="hall", name="hall"
                        )
                    else:
                        nc.vector.scalar_tensor_tensor(
                            c_new[:], st["m1"][:], 2.0, st["m2"][:], OP.mult, OP.add
                        )
                    st["c_prev"] = c_new
                return fn

            def mk_tanh(w, t):
                def fn():
                    st = state[w]
                    tanhc = apool.tile([128, WAVE], f16, tag="tanhc", name="tanhc")
                    nc.scalar.activation(tanhc[:], st["c_prev"][:], AF.Tanh)
                    st["tanhc"] = tanhc
                return fn

            def mk_h(w, t):
                def fn():
                    st = state[w]
                    h_new = st["h_all"][:, ts(t, WAVE)]
                    nc.vector.tensor_tensor(
                        h_new[:], st["sig"][:, 2 * WAVE : 3 * WAVE], st["tanhc"][:],
                        OP.mult,
                    )
                    st["h_new"] = h_new
                return fn

            def mk_dh(w, t):
                # DVE back-to-back after h: GpSimd's ~800ns TT plus two
                # cross-engine hops was the fattest slice of step latency
                def fn():
                    st = state[w]
                    dh = apool.tile([128, WAVE], f16, tag="dh", name="dh")
                    nc.vector.tensor_tensor(
                        dh[:], st["h_new"][:], st["h_prev"][:], OP.subtract
                    )
                    st["dh"] = dh
                    st["h_prev"] = st["h_new"]
                return fn

            def mk_mm(w, t):
                def fn():
                    st = state[w]
                    mv = st["h_new"] if t == 0 else st["dh"]
                    for g in range(4):
                        nc.tensor.matmul(
                            st["gates"][:, ts(g, WAVE)], whht_s[:, ts(g, 128)],
                            mv[:],
                            start=False, stop=(t == T - 2), skip_group_check=True,
                        )
                    if t == 0:
                        st["h_prev"] = st["h_new"]
                return fn

            # rel_pos: h-stationary matmuls (batch on out partitions,
            # N=2) into two small scratch PSUM tiles so the gates tile
            # frees at its last sigmoid; split copy across ACT/DVE.
            def mk_rel_mm(w, e0):
                def fn():
                    st = state[w]
                    blk = e0 // T
                    if e0 % T == 0:
                        st["relp%d" % blk] = scrpool.tile(
                            [128, 2 * T], f32, tag="scratch", name="relp"
                        )
                    rp = st["relp%d" % blk]
                    for e in range(e0, e0 + 6):
                        t = e % T
                        nc.tensor.matmul(
                            rp[:, 2 * t : 2 * t + 2],
                            st["h_all"][:, t * WAVE + blk * 128 :
                                         t * WAVE + blk * 128 + 128],
                            whpt_s[:], start=True, stop=True, skip_group_check=True,
                        )
                return fn

            def mk_rel_copy(w):
                # GPSIMD can't read PSUM; split the copy across ACT and DVE
                def fn():
                    st = state[w]
                    sb = opool.tile([128, 4 * T], f32, tag="predsb", name="predsb")
                    nc.scalar.copy(sb[:, 0 : 2 * T], st["relp0"][:])
                    nc.vector.tensor_copy(sb[:, 2 * T : 4 * T], st["relp1"][:])
                    st["predsb"] = sb
                return fn

            def mk_rel_dma(w):
                def fn():
                    st = state[w]
                    nc.sync.dma_start(pred[w], st["predsb"][:])
                return fn

            for w in range(NW):
                S = w * WS
                ev(S - 8000, mk_bias(w))
                for g in range(4):
                    ev(S - 7800 + 900 * g, mk_zxp(w, g))
                ev(S - 3600, mk_init_wes(w))
                ev(S - 2600, mk_init_hh(w))
                for t in range(T):
                    tau = S + t * L
                    ev(tau, mk_sig(w, t))
                    if t == 0:
                        ev(tau + 2700, mk_t0rm(w))
                    ev(tau + 1250, mk_m1(w, t))
                    if t > 0:
                        ev(tau + 1700, mk_m2(w, t))
                    ev(tau + 2150, mk_c(w, t))
                    ev(tau + 2550, mk_tanh(w, t))
                    ev(tau + 3100, mk_h(w, t))
                    if t == 0:
                        ev(tau + 3450, mk_mm(w, t))
                    elif t < T - 1:
                        ev(tau + 3450, mk_dh(w, t))
                        ev(tau + 3950, mk_mm(w, t))
                R = S + 11 * L
                for c in range(4):
                    ev(R + 3300 + 600 * c, mk_rel_mm(w, 6 * c))
                ev(R + 5700, mk_rel_copy(w))
                ev(R + 6100, mk_rel_dma(w))
            for p in range(NP):
                E = 2 * p * WS
                ev(-42000 if p == 0 else E - 27800, mk_zxw(p))
                for j in range(8):
                    u = E - 27000 + 2400 * j
                    for ci, k0 in enumerate((0, 3, 6)):
                        ev(u + 750 * ci, mk_mlp1_mm(p, j, k0))
                    ev(u + 2100, mk_mlp1_ts(p, j))
                for ci, (j0, nj) in enumerate(((0, 3), (3, 3), (6, 2))):
                    ev(E - 7600 + 700 * ci, mk_mlp2_mm(p, j0, nj))
                ev(E - 5800, mk_mlp2_ts(p))
            for jh in range(4):
                ev(-40600 + 300 * jh, mk_w1t_dma(jh))
            ev(-39400, mk_wzxt_dma(0))
            ev(-39100, mk_wzxt_dma(1))
            ev(-38800, mk_w2t_dma())

            for _, _, fn in sorted(events, key=lambda e: (e[0], e[1])):
                fn()

    nc.compile()
    return nc


def _prep(inputs):
    """Host-side weight folding + layout prep. Returns per-core input maps."""
    f = np.float64
    W_ih = np.asarray(inputs["W_ih"], f)
    W_hh = np.asarray(inputs["W_hh"], f)
    b_ih = np.asarray(inputs["b_ih"], f)
    b_hh = np.asarray(inputs["b_hh"], f)
    W1 = np.asarray(inputs["W1"], f)
    b1 = np.asarray(inputs["b1"], f)
    W2 = np.asarray(inputs["W2"], f)
    b2 = np.asarray(inputs["b2"], f)
    W_sp = np.asarray(inputs["W_sp"], f)
    b_sp = np.asarray(inputs["b_sp"], f)
    W_hp = np.asarray(inputs["W_hp"], f)
    b_hp = np.asarray(inputs["b_hp"], f)

    W_zx = W_ih[:, :ZX]
    W_emb = W_ih[:, ZX:]
    W_es = W_emb @ W_sp                       # [4H, 2]
    W_hh_f = W_hh + W_es @ W_hp               # [4H, H]
    bias1 = b_ih + b_hh + W_emb @ b_sp + W_es @ b_hp

    # reorder pytorch gates (i, f, g, o) -> (i, f, o, g)
    perm = np.r_[0:H, H : 2 * H, 3 * H : 4 * H, 2 * H : 3 * H]
    W_zx = W_zx[perm]
    W_hh_f = W_hh_f[perm]
    W_es = W_es[perm]
    bias1 = bias1[perm]
    # double the g-gate block: its bank then holds 2*g_pre, so
    # tanh(g) = 2*sigmoid(2*g_pre) - 1 comes out of the one big sigmoid
    dbl = np.ones((G4, 1))
    dbl[3 * H :] = 2.0
    W_zx = W_zx * dbl
    W_hh_f = W_hh_f * dbl
    W_es = W_es * dbl
    bias1 = bias1 * dbl[:, 0]

    def kxm(Wt, kp):  # [K, M] -> [128, K/128, M] fp16, zero-padded to kp rows
        K, M = Wt.shape
        out = np.zeros((kp, M), f)
        out[:K] = Wt
        return np.ascontiguousarray(
            out.reshape(kp // 128, 128, M).transpose(1, 0, 2)
        ).astype(np.float16)

    W_hh_pd = W_hh[perm] * dbl
    consts = {
        "whh0": np.ascontiguousarray(W_hh_pd.T).astype(np.float16),
        "whhn": np.ascontiguousarray(-W_hh_pd.T).astype(np.float16),
        "w1t": kxm(W1.T, KP),
        "wzxt": kxm(W_zx.T, KP),
        "w2t": kxm(W2.T, MLP),
        "whht": np.ascontiguousarray(W_hh_f.T).astype(np.float16),
        "whpt": np.ascontiguousarray(W_hp.T).astype(np.float16),
        "k3": np.ascontiguousarray(-W_es.T).astype(np.float16),
        "wes": np.ascontiguousarray(W_es.T).astype(np.float16),
        # bank-open bias: bias2[r, bk*128+m] = bias1[(2*bk + r)*128 + m]
        "bias2": np.ascontiguousarray(
            bias1.reshape(4, 128).reshape(2, 2, 128).transpose(1, 0, 2).reshape(2, 256)
        ).astype(np.float16),
        # 0/1 indicator selecting which half-bank gets which bias row
        "ind": np.kron(np.eye(2), np.ones((1, WAVE))).astype(np.float16),
        "b1": np.ascontiguousarray(b1.reshape(8, 128).T).astype(np.float32),
        "b2": b2.reshape(128, 1).astype(np.float32),
    }

    enc = np.asarray(inputs["enc_h_feat"], np.float32)
    z = np.asarray(inputs["z"], np.float32)
    lpr = np.asarray(inputs["last_pos_rel"], np.float32)
    zxT = np.zeros((KP, B), np.float16)
    zxT[:MLP] = enc.T
    zxT[MLP:ZX] = z.T
    lprT = np.ascontiguousarray((lpr - b_hp[None, :]).T).astype(np.float16)

    in_maps = []
    for c in range(NCORES):
        s = slice(c * BC, (c + 1) * BC)
        m = dict(consts)
        m["zxT"] = np.ascontiguousarray(zxT[:, s])
        m["lprT"] = np.ascontiguousarray(lprT[:, s])
        in_maps.append(m)
    return in_maps


def run(inputs, trace=False):
    from concourse.bass_utils import run_bass_kernel_spmd

    if "nc" not in _cache:
        _cache["nc"] = _build_nc()
    in_maps = _prep(inputs)
    res = run_bass_kernel_spmd(
        _cache["nc"], in_maps, core_ids=list(range(NCORES)), trace=trace
    )
    # per core: [NW, 128, (blk, t, j)]; batch = w*256 + blk*128 + p
    def decode(a):
        a = a.reshape(NW, 128, 2, T, 2)              # w p blk t j
        return a.transpose(0, 2, 1, 3, 4).reshape(BC, T, 2)
    pred = np.concatenate(
        [decode(r["pred"]) for r in res.results], axis=0
    )  # [B, T, 2]
    out = pred.transpose(1, 0, 2) + np.asarray(inputs["b_hp"], np.float32)[None, None, :]
    return np.ascontiguousarray(out), res


def kernel(**inputs) -> np.ndarray:
    out, _ = run(inputs, trace=False)
    return out



# revision 25
# speedup vs baseline: 1.0247x; 1.0144x over previous
"""Trainium2 Bass kernel for the trajectory-decoder LSTM problem.

Math (mirrors the reference, with algebraic folds):
  dec_inp feeds gates only through W_emb; W_sp/W_emb/W_hp collapse:
    W_es = W_emb @ W_sp            [4H, 2]
    gates_t = zx@W_zx.T + bias + r_{t-1}@W_es.T + h_{t-1}@W_hh.T
  For t>=1, r_{t-1} = h_{t-1}@W_hp.T + b_hp, so with
    W_hh' = W_hh + W_es @ W_hp,  bias1 = b_ih + b_hh + W_emb@b_sp + W_es@b_hp
  every step becomes uniform:  gates_t = zx@W_zx.T + bias1 + h_{t-1}@W_hh'.T
  The step-0 state needs no r_init projection: since the W_es@b_hp term
  of bias1 cancels,  gates_0 = zxp + bias1 + h_init@W_hh.T
                               + (lpr - b_hp)@W_es.T
  and the step-0 update adds h_0@W_hh'.T - h_init@W_hh.T - lprm@W_es.T,
  whose last two terms depend only on consts + h_init and issue right
  after sigmoid_0 (off the elementwise critical path).
  `last_pos` is dead code (output is just the stacked rel_pos).

Device strategy (pure data-parallel over 8 cores, 4096 batch each):
  - batch on the free dim, features on partitions
  - per 256-batch wave, the 4 gate pre-activations live RESIDENT in one
    2-bank PSUM tile; each step the PE accumulates (h_t - h_{t-1}) @ W_hh'.T
    into it (start=False), so no per-step zxp re-add on the vector engine.
    A K=2 "bank-open" matmul (bias x 0/1 indicator) clears has_written for
    the whole bank and injects the per-gate bias.
  - the g-gate weights are doubled on host so ONE sigmoid over all 4 banks
    yields sig(i),sig(f),sig(o),sig(2g); tanh(g)=2*sig(2g)-1 folds into the
    DVE scalar_tensor_tensor ops for m1 and c.
  - per-step elementwise: sigmoid (ACT), then m1/m2/c and h/dh as
    back-to-back DVE ops (same-queue chaining avoids cross-engine
    semaphore hops on the critical path); tanh on ACT.
  - rel_pos: h-stationary matmuls (batch on out partitions, N=2) into
    small scratch PSUM tiles, copied out via ACT+DVE, one contiguous
    partition-major DMA per wave.
  - waves are software-pipelined on a virtual timeline: every op gets
    an estimated execution time tau and ops are emitted sorted by tau,
    so each engine's in-order queue matches true readiness order.
    Chains step every L=5us; waves start every WS=23.4us (PE-
    feasibility bound). Filler matmuls (mlp/init) are chopped into
    <=0.7us chunks; weight DMAs go on the idle GpSimd queue, chunked
    so pair 0's first mlp unit starts after ~0.7MB.
  - all matmul operands fp16 (full PE rate), PSUM accumulation fp32;
    c kept in fp16, b_hp added on host after gather.
"""

import os
import numpy as np

B = 32768
NCORES = 8
BC = B // NCORES          # 4096 batch per core
WAVE = 256                # batch per recurrence chain (2 PSUM banks of gates)
NW = BC // WAVE           # 16 waves
PAIR = 2 * WAVE           # phase-A (mlp) runs at N=512 across wave pairs
T = 12                    # decode steps
H = 128
G4 = 4 * H                # 512 gate features
ZX = 1056
KP = 1152                 # ZX padded to 9*128
KT = KP // 128            # 9 contraction tiles
MLP = 1024
EMB = 64

_cache = {}


def _build_nc():
    import concourse.bass as bass
    import concourse.bacc as bacc
    import concourse.mybir as mybir
    import concourse.tile as tile
    from concourse.bass import ts

    f16 = mybir.dt.float16
    f32 = mybir.dt.float32
    AF = mybir.ActivationFunctionType
    OP = mybir.AluOpType

    nc = bacc.Bacc("TRN2", target_bir_lowering=False)

    zxT = nc.dram_tensor("zxT", [KP, BC], f16, kind="ExternalInput")
    lprT = nc.dram_tensor("lprT", [2, BC], f16, kind="ExternalInput")
    w1t = nc.dram_tensor("w1t", [128, KT, MLP], f16, kind="ExternalInput")
    wzxt = nc.dram_tensor("wzxt", [128, KT, G4], f16, kind="ExternalInput")
    w2t = nc.dram_tensor("w2t", [128, 8, H], f16, kind="ExternalInput")
    whht = nc.dram_tensor("whht", [128, G4], f16, kind="ExternalInput")
    whpt = nc.dram_tensor("whpt", [128, 2], f16, kind="ExternalInput")
    k3 = nc.dram_tensor("k3", [2, G4], f16, kind="ExternalInput")   # -W_es.T
    wes = nc.dram_tensor("wes", [2, G4], f16, kind="ExternalInput")  # +W_es.T
    bias2 = nc.dram_tensor("bias2", [2, 2 * 128], f16, kind="ExternalInput")
    ind = nc.dram_tensor("ind", [2, 2 * WAVE], f16, kind="ExternalInput")
    bias4 = nc.dram_tensor("bias4", [4, 128], f16, kind="ExternalInput")
    ind4 = nc.dram_tensor("ind4", [4, 512], f16, kind="ExternalInput")
    b1 = nc.dram_tensor("b1", [128, 8], f32, kind="ExternalInput")
    b2 = nc.dram_tensor("b2", [128, 1], f32, kind="ExternalInput")
    whh0 = nc.dram_tensor("whh0", [128, G4], f16, kind="ExternalInput")
    whhn = nc.dram_tensor("whhn", [128, G4], f16, kind="ExternalInput")
    # per wave: [partition=batch, (blk, t, j)] — partition-major so the
    # rel output DMA is one clean 192B-per-partition descriptor
    pred = nc.dram_tensor("pred", [NW, 128, 4 * T], f32, kind="ExternalOutput")

    with tile.TileContext(nc) as tc:
        with (
            tc.tile_pool(name="consts", bufs=1) as cpool,
            tc.tile_pool(name="zx", bufs=2) as zxpool,
            tc.tile_pool(name="h1", bufs=2) as h1pool,
            tc.tile_pool(name="hc", bufs=10) as hcpool,
            tc.tile_pool(name="acts", bufs=6) as apool,
            tc.tile_pool(name="outs", bufs=3) as opool,
            tc.tile_pool(name="scrps", bufs=2, space="PSUM") as scrpool,
            tc.tile_pool(name="gateps", bufs=3, space="PSUM") as gatepool,
        ):
            # ---- constants: small ones loaded upfront; the big mlp/zxp
            # weights are DMA'd in chunks as tau-events so pair 0's first
            # mlp unit starts after ~0.7MB instead of ~5MB ----
            w1t_s = cpool.tile([128, KT, MLP], f16)
            wzxt_s = cpool.tile([128, KT, G4], f16)
            w2t_s = cpool.tile([128, 8, H], f16)
            whht_s = cpool.tile([128, G4], f16)
            nc.sync.dma_start(whht_s[:], whht[:])
            whpt_s = cpool.tile([128, 2], f16)
            nc.sync.dma_start(whpt_s[:], whpt[:])
            k3_s = cpool.tile([2, G4], f16)
            nc.sync.dma_start(k3_s[:], k3[:])
            wes_s = cpool.tile([2, G4], f16)
            nc.sync.dma_start(wes_s[:], wes[:])
            bias2_s = cpool.tile([2, 2 * 128], f16)
            nc.sync.dma_start(bias2_s[:], bias2[:])
            ind_s = cpool.tile([2, 2 * WAVE], f16)
            nc.sync.dma_start(ind_s[:], ind[:])
            bias4_s = cpool.tile([4, 128], f16)
            nc.sync.dma_start(bias4_s[:], bias4[:])
            ind4_s = cpool.tile([4, 512], f16)
            nc.sync.dma_start(ind4_s[:], ind4[:])
            b1_s = cpool.tile([128, 8], f32)
            nc.sync.dma_start(b1_s[:], b1[:])
            b2_s = cpool.tile([128, 1], f32)
            nc.sync.dma_start(b2_s[:], b2[:])
            whh0_s = cpool.tile([128, G4], f16)
            nc.sync.dma_start(whh0_s[:], whh0[:])
            whhn_s = cpool.tile([128, G4], f16)
            nc.sync.dma_start(whhn_s[:], whhn[:])
            lpr_s = cpool.tile([2, BC], f16)
            nc.sync.dma_start(lpr_s[:], lprT[:])

            zxT_v = zxT.rearrange("(k p) b -> p k b", p=128)

            # Virtual-timeline schedule: every op gets an estimated
            # execution time tau (ns) and ops are emitted sorted by tau,
            # so each engine's in-order queue matches true readiness
            # order. Chains step every L; waves start every WS (set by
            # PE feasibility: ~22us of PE work per wave). Filler matmul
            # bursts (mlp/init/rel) are chopped into <=0.7us chunks so a
            # chain's gate matmuls are never queued behind a long burst.
            NP = NW // 2
            state = [dict() for _ in range(NW)]
            pair_state = [dict() for _ in range(NP)]
            events = []

            L = 5000
            WS = 23400

            def ev(tau, fn):
                events.append((tau, len(events), fn))

            def mk_w1t_dma(jh):
                def fn():
                    eng = nc.gpsimd
                    eng.dma_start(
                        w1t_s[:, :, ts(jh, 256)], w1t[:, :, ts(jh, 256)]
                    )
                return fn

            def mk_wzxt_dma(half):
                def fn():
                    nc.gpsimd.dma_start(
                        wzxt_s[:, :, ts(half, 256)], wzxt[:, :, ts(half, 256)]
                    )
                return fn

            def mk_w2t_dma():
                def fn():
                    nc.sync.dma_start(w2t_s[:], w2t[:])
                return fn

            def mk_zxw(p):
                def fn():
                    st = pair_state[p]
                    zxw = zxpool.tile([128, KT, PAIR], f16, tag="zxw", name="zxw")
                    nc.gpsimd.dma_start(zxw[:], zxT_v[:, :, ts(p, PAIR)])
                    st["zxw"] = zxw
                    st["h1"] = h1pool.tile([128, 8, PAIR], f16, tag="h1", name="h1")
                return fn

            def mk_mlp1_mm(p, j, k0):
                def fn():
                    st = pair_state[p]
                    if k0 == 0:
                        st["ps"] = scrpool.tile([128, PAIR], f32, tag="scratch", name="ps")
                    for k in range(k0, min(k0 + 3, KT)):
                        nc.tensor.matmul(
                            st["ps"][:], w1t_s[:, k, ts(j, 128)], st["zxw"][:, k, :],
                            start=(k == 0), stop=(k == KT - 1),
                        )
                return fn

            def mk_mlp1_ts(p, j):
                def fn():
                    st = pair_state[p]
                    if j % 2 == 0:
                        nc.scalar.activation(
                            st["h1"][:, j, :], st["ps"][:], AF.Relu,
                            bias=b1_s[:, j : j + 1],
                        )
                    else:
                        nc.vector.tensor_scalar(
                            st["h1"][:, j, :], st["ps"][:], b1_s[:, j : j + 1],
                            0.0, OP.add, OP.max,
                        )
                return fn

            def mk_mlp2_mm(p, j0, nj):
                def fn():
                    st = pair_state[p]
                    if j0 == 0:
                        st["ps"] = scrpool.tile([128, PAIR], f32, tag="scratch", name="ps")
                    for j in range(j0, j0 + nj):
                        nc.tensor.matmul(
                            st["ps"][:], w2t_s[:, j, :], st["h1"][:, j, :],
                            start=(j == 0), stop=(j == 7),
                        )
                return fn

            def mk_mlp2_ts(p):
                def fn():
                    st = pair_state[p]
                    hi = h1pool.tile([128, PAIR], f16, tag="hinit", name="hinit")
                    nc.vector.tensor_scalar(
                        hi[:], st["ps"][:], b2_s[:, 0:1], 0.0, OP.add, OP.max
                    )
                    st["h_init"] = hi
                return fn

            def mk_bias(w):
                def fn():
                    st = state[w]
                    gates = gatepool.tile([128, 4 * WAVE], f32, tag="gates", name="gates")
                    st["gates"] = gates
                    for bk in range(2):
                        nc.tensor.matmul(
                            gates[:, ts(bk, 2 * WAVE)], bias2_s[:, ts(bk, 128)],
                            ind_s[:], start=True, stop=False, skip_group_check=True,
                        )
                return fn

            def mk_zxp(w, g):
                def fn():
                    st = state[w]
                    pst = pair_state[w // 2]
                    hs = ts(w % 2, WAVE)
                    gp = st["gates"][:, ts(g, WAVE)]
                    for k in range(KT):
                        nc.tensor.matmul(
                            gp[:], wzxt_s[:, k, ts(g, 128)], pst["zxw"][:, k, hs],
                            start=False, stop=False, skip_group_check=True,
                        )
                return fn

            # gates_0 = zxp + bias + h_init@W_hh.T + (lpr-b_hp)@W_es.T
            # (the W_es@b_hp term of bias1 cancels; no r_init projection
            # needed, so the pair handoff is just mlp2 -> init_hh -> sig)
            def mk_init_wes(w):
                def fn():
                    st = state[w]
                    for g in range(4):
                        nc.tensor.matmul(
                            st["gates"][:, ts(g, WAVE)], wes_s[:, ts(g, 128)],
                            lpr_s[:, ts(w, WAVE)],
                            start=False, stop=False, skip_group_check=True,
                        )
                return fn

            def mk_init_hh(w):
                def fn():
                    st = state[w]
                    st["h_prev"] = pair_state[w // 2]["h_init"][:, ts(w % 2, WAVE)]
                    for g in range(4):
                        nc.tensor.matmul(
                            st["gates"][:, ts(g, WAVE)], whh0_s[:, ts(g, 128)],
                            st["h_prev"][:],
                            start=False, stop=False, skip_group_check=True,
                        )
                return fn

            # step-0 removal: gates += -h_init@W_hh.T - (lpr-b_hp)@W_es.T;
            # depends only on consts + h_init, issues right after sigmoid_0
            def mk_t0rm(w):
                def fn():
                    st = state[w]
                    for g in range(4):
                        gp = st["gates"][:, ts(g, WAVE)]
                        nc.tensor.matmul(
                            gp[:], whhn_s[:, ts(g, 128)], st["h_prev"][:],
                            start=False, stop=False, skip_group_check=True,
                        )
                        nc.tensor.matmul(
                            gp[:], k3_s[:, ts(g, 128)], lpr_s[:, ts(w, WAVE)],
                            start=False, stop=False, skip_group_check=True,
                        )
                return fn

            def mk_sig(w, t):
                def fn():
                    st = state[w]
                    sig = apool.tile([128, 4 * WAVE], f16, tag="sig", name="sig")
                    nc.scalar.activation(sig[:], st["gates"][:], AF.Sigmoid)
                    st["sig"] = sig
                return fn

            def mk_m1(w, t):
                def fn():
                    st = state[w]
                    m1 = apool.tile([128, WAVE], f16, tag="m1", name="m1")
                    nc.vector.scalar_tensor_tensor(
                        m1[:], st["sig"][:, 3 * WAVE : 4 * WAVE], 0.5,
                        st["sig"][:, 0:WAVE], OP.subtract, OP.mult,
                    )
                    st["m1"] = m1
                return fn

            def mk_m2(w, t):
                # on DVE right after m1: same-queue back-to-back beats the
                # two cross-engine hops of a Pool m2 on the c critical path
                def fn():
                    st = state[w]
                    m2 = apool.tile([128, WAVE], f16, tag="m2", name="m2")
                    nc.vector.tensor_tensor(
                        m2[:], st["sig"][:, WAVE : 2 * WAVE], st["c_prev"][:], OP.mult
                    )
                    st["m2"] = m2
                return fn

            def mk_c(w, t):
                def fn():
                    st = state[w]
                    c_new = hcpool.tile([128, WAVE], f16, tag="c", name="c")
                    if t == 0:
                        nc.vector.tensor_scalar_mul(c_new[:], st["m1"][:], 2.0)
                        st["h_all"] = opool.tile(
                            [128, T * WAVE], f16, tag="hall", name="hall"
                        )
                    else:
                        nc.vector.scalar_tensor_tensor(
                            c_new[:], st["m1"][:], 2.0, st["m2"][:], OP.mult, OP.add
                        )
                    st["c_prev"] = c_new
                return fn

            def mk_tanh(w, t):
                def fn():
                    st = state[w]
                    tanhc = apool.tile([128, WAVE], f16, tag="tanhc", name="tanhc")
                    nc.scalar.activation(tanhc[:], st["c_prev"][:], AF.Tanh)
                    st["tanhc"] = tanhc
                return fn

            def mk_h(w, t):
                def fn():
                    st = state[w]
                    h_new = st["h_all"][:, ts(t, WAVE)]
                    nc.vector.tensor_tensor(
                        h_new[:], st["sig"][:, 2 * WAVE : 3 * WAVE], st["tanhc"][:],
                        OP.mult,
                    )
                    st["h_new"] = h_new
                return fn

            def mk_dh(w, t):
                # DVE back-to-back after h: GpSimd's ~800ns TT plus two
                # cross-engine hops was the fattest slice of step latency
                def fn():
                    st = state[w]
                    dh = apool.tile([128, WAVE], f16, tag="dh", name="dh")
                    nc.vector.tensor_tensor(
                        dh[:], st["h_new"][:], st["h_prev"][:], OP.subtract
                    )
                    st["dh"] = dh
                    st["h_prev"] = st["h_new"]
                return fn

            def mk_mm(w, t):
                def fn():
                    st = state[w]
                    mv = st["h_new"] if t == 0 else st["dh"]
                    for g in range(4):
                        nc.tensor.matmul(
                            st["gates"][:, ts(g, WAVE)], whht_s[:, ts(g, 128)],
                            mv[:],
                            start=False, stop=(t == T - 2), skip_group_check=True,
                        )
                    if t == 0:
                        st["h_prev"] = st["h_new"]
                return fn

            # rel_pos: h-stationary matmuls (batch on out partitions,
            # N=2) into two small scratch PSUM tiles so the gates tile
            # frees at its last sigmoid; split copy across ACT/DVE.
            def mk_rel_mm(w, e0):
                def fn():
                    st = state[w]
                    blk = e0 // T
                    if e0 % T == 0:
                        st["relp%d" % blk] = scrpool.tile(
                            [128, 2 * T], f32, tag="scratch", name="relp"
                        )
                    rp = st["relp%d" % blk]
                    for e in range(e0, e0 + 6):
                        t = e % T
                        nc.tensor.matmul(
                            rp[:, 2 * t : 2 * t + 2],
                            st["h_all"][:, t * WAVE + blk * 128 :
                                         t * WAVE + blk * 128 + 128],
                            whpt_s[:], start=True, stop=True, skip_group_check=True,
                        )
                return fn

            def mk_rel_copy(w):
                # GPSIMD can't read PSUM; split the copy across ACT and DVE
                def fn():
                    st = state[w]
                    sb = opool.tile([128, 4 * T], f32, tag="predsb", name="predsb")
                    nc.scalar.copy(sb[:, 0 : 2 * T], st["relp0"][:])
                    nc.vector.tensor_copy(sb[:, 2 * T : 4 * T], st["relp1"][:])
                    st["predsb"] = sb
                return fn

            def mk_rel_dma(w):
                def fn():
                    st = state[w]
                    nc.sync.dma_start(pred[w], st["predsb"][:])
                return fn

            # --- last wave split into two concurrent 128-wide chains:
            # the tail is latency-bound, and a 128-wide chain's natural
            # step latency is ~2.7us vs ~4.6us at 256 ---
            tst = [dict(), dict()]

            def tk_bias(h):
                def fn():
                    st = tst[h]
                    if h == 0:
                        g = gatepool.tile([128, 4 * WAVE], f32, tag="gates",
                                          name="gates")
                        tst[0]["gt"] = tst[1]["gt"] = g
                    st["gates"] = st["gt"][:, ts(h, 512)]
                    nc.tensor.matmul(
                        st["gates"][:], bias4_s[:], ind4_s[:],
                        start=True, stop=False, skip_group_check=True,
                    )
                return fn

            def tk_zxp(h, g):
                def fn():
                    st = tst[h]
                    pst = pair_state[NP - 1]
                    hs = slice(256 + 128 * h, 384 + 128 * h)
                    for k in range(KT):
                        nc.tensor.matmul(
                            st["gates"][:, ts(g, 128)],
                            wzxt_s[:, k, ts(g, 128)], pst["zxw"][:, k, hs],
                            start=False, stop=False, skip_group_check=True,
                        )
                return fn

            def tk_init(h):
                def fn():
                    st = tst[h]
                    st["h_prev"] = pair_state[NP - 1]["h_init"][
                        :, 256 + 128 * h : 384 + 128 * h]
                    for g in range(4):
                        nc.tensor.matmul(
                            st["gates"][:, ts(g, 128)], wes_s[:, ts(g, 128)],
                            lpr_s[:, BC - 256 + 128 * h : BC - 128 + 128 * h],
                            start=False, stop=False, skip_group_check=True,
                        )
                        nc.tensor.matmul(
                            st["gates"][:, ts(g, 128)], whh0_s[:, ts(g, 128)],
                            st["h_prev"][:],
                            start=False, stop=False, skip_group_check=True,
                        )
                return fn

            def tk_t0rm(h):
                def fn():
                    st = tst[h]
                    for g in range(4):
                        nc.tensor.matmul(
                            st["gates"][:, ts(g, 128)], whhn_s[:, ts(g, 128)],
                            st["h_prev"][:],
                            start=False, stop=False, skip_group_check=True,
                        )
                        nc.tensor.matmul(
                            st["gates"][:, ts(g, 128)], k3_s[:, ts(g, 128)],
                            lpr_s[:, BC - 256 + 128 * h : BC - 128 + 128 * h],
                            start=False, stop=False, skip_group_check=True,
                        )
                return fn

            def tk_sig(h, t):
                def fn():
                    st = tst[h]
                    sig = apool.tile([128, 512], f16, tag="sigS", name="sigS")
                    nc.scalar.activation(sig[:], st["gates"][:], AF.Sigmoid)
                    st["sig"] = sig
                return fn

            def tk_m1(h, t):
                def fn():
                    st = tst[h]
                    m1 = apool.tile([128, 128], f16, tag="m1S", name="m1S")
                    nc.vector.scalar_tensor_tensor(
                        m1[:], st["sig"][:, 384:512], 0.5,
                        st["sig"][:, 0:128], OP.subtract, OP.mult,
                    )
                    st["m1"] = m1
                return fn

            def tk_m2(h, t):
                def fn():
                    st = tst[h]
                    m2 = apool.tile([128, 128], f16, tag="m2S", name="m2S")
                    nc.vector.tensor_tensor(
                        m2[:], st["sig"][:, 128:256], st["c_prev"][:], OP.mult
                    )
                    st["m2"] = m2
                return fn

            def tk_c(h, t):
                def fn():
                    st = tst[h]
                    c_new = hcpool.tile([128, 128], f16, tag="cS", name="cS")
                    if t == 0:
                        nc.vector.tensor_scalar_mul(c_new[:], st["m1"][:], 2.0)
                        st["h_all"] = opool.tile(
                            [128, T * 128], f16, tag="hallS", name="hallS"
                        )
                    else:
                        nc.vector.scalar_tensor_tensor(
                            c_new[:], st["m1"][:], 2.0, st["m2"][:],
                            OP.mult, OP.add,
                        )
                    st["c_prev"] = c_new
                return fn

            def tk_tanh(h, t):
                def fn():
                    st = tst[h]
                    tanhc = apool.tile([128, 128], f16, tag="tanhcS", name="tanhcS")
                    nc.scalar.activation(tanhc[:], st["c_prev"][:], AF.Tanh)
                    st["tanhc"] = tanhc
                return fn

            def tk_h(h, t):
                def fn():
                    st = tst[h]
                    h_new = st["h_all"][:, ts(t, 128)]
                    nc.vector.tensor_tensor(
                        h_new[:], st["sig"][:, 256:384], st["tanhc"][:], OP.mult
                    )
                    st["h_new"] = h_new
                return fn

            def tk_dh(h, t):
                def fn():
                    st = tst[h]
                    dh = apool.tile([128, 128], f16, tag="dhS", name="dhS")
                    nc.vector.tensor_tensor(
                        dh[:], st["h_new"][:], st["h_prev"][:], OP.subtract
                    )
                    st["dh"] = dh
                    st["h_prev"] = st["h_new"]
                return fn

            def tk_mm(h, t):
                def fn():
                    st = tst[h]
                    mv = st["h_new"] if t == 0 else st["dh"]
                    for g in range(4):
                        nc.tensor.matmul(
                            st["gates"][:, ts(g, 128)], whht_s[:, ts(g, 128)],
                            mv[:],
                            start=False, stop=(t == T - 2), skip_group_check=True,
                        )
                    if t == 0:
                        st["h_prev"] = st["h_new"]
                return fn

            def tk_rel_mm(h, e0):
                def fn():
                    st = tst[h]
                    if e0 == 0:
                        st["relp"] = scrpool.tile(
                            [128, 2 * T], f32, tag="scratch", name="relp"
                        )
                    for e in range(e0, e0 + 6):
                        nc.tensor.matmul(
                            st["relp"][:, 2 * e : 2 * e + 2],
                            st["h_all"][:, ts(e, 128)], whpt_s[:],
                            start=True, stop=True, skip_group_check=True,
                        )
                return fn

            def tk_rel_copy(h):
                def fn():
                    st = tst[h]
                    if h == 0:
                        sb = opool.tile([128, 4 * T], f32, tag="predsb",
                                        name="predsb")
                        tst[0]["sb"] = tst[1]["sb"] = sb
                        nc.scalar.copy(sb[:, 0 : 2 * T], st["relp"][:])
                    else:
                        nc.vector.tensor_copy(
                            st["sb"][:, 2 * T : 4 * T], st["relp"][:]
                        )
                return fn

            def tk_rel_dma():
                def fn():
                    nc.sync.dma_start(pred[NW - 1], tst[0]["sb"][:])
                return fn

            for w in range(NW):
                S = w * WS
                if w == NW - 1:
                    LT = 3200
                    for h in range(2):
                        hb = 1600 * h
                        ev(S - 8000 + 100 * h, tk_bias(h))
                        for g in range(4):
                            ev(S - 7800 + 450 * (g + 4 * h), tk_zxp(h, g))
                        ev(S - 3600 + 150 * h, tk_init(h))
                        for t in range(T):
                            tau = S + t * LT + hb
                            ev(tau, tk_sig(h, t))
                            if t == 0:
                                ev(tau + 1730, tk_t0rm(h))
                            ev(tau + 800, tk_m1(h, t))
                            if t > 0:
                                ev(tau + 1090, tk_m2(h, t))
                            ev(tau + 1375, tk_c(h, t))
                            ev(tau + 1630, tk_tanh(h, t))
                            ev(tau + 1980, tk_h(h, t))
                            if t < T - 1:
                                ev(tau + 2205, tk_dh(h, t))
                                ev(tau + 2525, tk_mm(h, t))
                        R = S + 11 * LT + hb
                        ev(R + 2200 + 300 * h, tk_rel_mm(h, 0))
                        ev(R + 2500 + 300 * h, tk_rel_mm(h, 6))
                        ev(R + 3100 + 300 * h, tk_rel_copy(h))
                    ev(S + 11 * LT + 1600 + 3700, tk_rel_dma())
                    continue
                ev(S - 8000, mk_bias(w))
                for g in range(4):
                    ev(S - 7800 + 900 * g, mk_zxp(w, g))
                ev(S - 3600, mk_init_wes(w))
                ev(S - 2600, mk_init_hh(w))
                for t in range(T):
                    tau = S + t * L
                    ev(tau, mk_sig(w, t))
                    if t == 0:
                        ev(tau + 2700, mk_t0rm(w))
                    ev(tau + 1250, mk_m1(w, t))
                    if t > 0:
                        ev(tau + 1700, mk_m2(w, t))
                    ev(tau + 2150, mk_c(w, t))
                    ev(tau + 2550, mk_tanh(w, t))
                    ev(tau + 3100, mk_h(w, t))
                    if t == 0:
                        ev(tau + 3450, mk_mm(w, t))
                    elif t < T - 1:
                        ev(tau + 3450, mk_dh(w, t))
                        ev(tau + 3950, mk_mm(w, t))
                R = S + 11 * L
                for c in range(4):
                    ev(R + 3300 + 600 * c, mk_rel_mm(w, 6 * c))
                ev(R + 5700, mk_rel_copy(w))
                ev(R + 6100, mk_rel_dma(w))
            for p in range(NP):
                E = 2 * p * WS
                ev(-42000 if p == 0 else E - 27800, mk_zxw(p))
                for j in range(8):
                    u = E - 27000 + 2400 * j
                    for ci, k0 in enumerate((0, 3, 6)):
                        ev(u + 750 * ci, mk_mlp1_mm(p, j, k0))
                    ev(u + 2100, mk_mlp1_ts(p, j))
                for ci, (j0, nj) in enumerate(((0, 3), (3, 3), (6, 2))):
                    ev(E - 7600 + 700 * ci, mk_mlp2_mm(p, j0, nj))
                ev(E - 5800, mk_mlp2_ts(p))
            for jh in range(4):
                ev(-40600 + 300 * jh, mk_w1t_dma(jh))
            ev(-39400, mk_wzxt_dma(0))
            ev(-39100, mk_wzxt_dma(1))
            ev(-38800, mk_w2t_dma())

            for _, _, fn in sorted(events, key=lambda e: (e[0], e[1])):
                fn()

    nc.compile()
    return nc


def _prep(inputs):
    """Host-side weight folding + layout prep. Returns per-core input maps."""
    f = np.float64
    W_ih = np.asarray(inputs["W_ih"], f)
    W_hh = np.asarray(inputs["W_hh"], f)
    b_ih = np.asarray(inputs["b_ih"], f)
    b_hh = np.asarray(inputs["b_hh"], f)
    W1 = np.asarray(inputs["W1"], f)
    b1 = np.asarray(inputs["b1"], f)
    W2 = np.asarray(inputs["W2"], f)
    b2 = np.asarray(inputs["b2"], f)
    W_sp = np.asarray(inputs["W_sp"], f)
    b_sp = np.asarray(inputs["b_sp"], f)
    W_hp = np.asarray(inputs["W_hp"], f)
    b_hp = np.asarray(inputs["b_hp"], f)

    W_zx = W_ih[:, :ZX]
    W_emb = W_ih[:, ZX:]
    W_es = W_emb @ W_sp                       # [4H, 2]
    W_hh_f = W_hh + W_es @ W_hp               # [4H, H]
    bias1 = b_ih + b_hh + W_emb @ b_sp + W_es @ b_hp

    # reorder pytorch gates (i, f, g, o) -> (i, f, o, g)
    perm = np.r_[0:H, H : 2 * H, 3 * H : 4 * H, 2 * H : 3 * H]
    W_zx = W_zx[perm]
    W_hh_f = W_hh_f[perm]
    W_es = W_es[perm]
    bias1 = bias1[perm]
    # double the g-gate block: its bank then holds 2*g_pre, so
    # tanh(g) = 2*sigmoid(2*g_pre) - 1 comes out of the one big sigmoid
    dbl = np.ones((G4, 1))
    dbl[3 * H :] = 2.0
    W_zx = W_zx * dbl
    W_hh_f = W_hh_f * dbl
    W_es = W_es * dbl
    bias1 = bias1 * dbl[:, 0]

    def kxm(Wt, kp):  # [K, M] -> [128, K/128, M] fp16, zero-padded to kp rows
        K, M = Wt.shape
        out = np.zeros((kp, M), f)
        out[:K] = Wt
        return np.ascontiguousarray(
            out.reshape(kp // 128, 128, M).transpose(1, 0, 2)
        ).astype(np.float16)

    W_hh_pd = W_hh[perm] * dbl
    consts = {
        "whh0": np.ascontiguousarray(W_hh_pd.T).astype(np.float16),
        "whhn": np.ascontiguousarray(-W_hh_pd.T).astype(np.float16),
        "w1t": kxm(W1.T, KP),
        "wzxt": kxm(W_zx.T, KP),
        "w2t": kxm(W2.T, MLP),
        "whht": np.ascontiguousarray(W_hh_f.T).astype(np.float16),
        "whpt": np.ascontiguousarray(W_hp.T).astype(np.float16),
        "k3": np.ascontiguousarray(-W_es.T).astype(np.float16),
        "wes": np.ascontiguousarray(W_es.T).astype(np.float16),
        # bank-open bias: bias2[r, bk*128+m] = bias1[(2*bk + r)*128 + m]
        "bias2": np.ascontiguousarray(
            bias1.reshape(4, 128).reshape(2, 2, 128).transpose(1, 0, 2).reshape(2, 256)
        ).astype(np.float16),
        # 0/1 indicator selecting which half-bank gets which bias row
        "ind": np.kron(np.eye(2), np.ones((1, WAVE))).astype(np.float16),
        # K=4 open for the 128-wide tail half-waves (4 gate blocks per bank)
        "bias4": np.ascontiguousarray(bias1.reshape(4, 128)).astype(np.float16),
        "ind4": np.kron(np.eye(4), np.ones((1, 128))).astype(np.float16),
        "b1": np.ascontiguousarray(b1.reshape(8, 128).T).astype(np.float32),
        "b2": b2.reshape(128, 1).astype(np.float32),
    }

    enc = np.asarray(inputs["enc_h_feat"], np.float32)
    z = np.asarray(inputs["z"], np.float32)
    lpr = np.asarray(inputs["last_pos_rel"], np.float32)
    zxT = np.zeros((KP, B), np.float16)
    zxT[:MLP] = enc.T
    zxT[MLP:ZX] = z.T
    lprT = np.ascontiguousarray((lpr - b_hp[None, :]).T).astype(np.float16)

    in_maps = []
    for c in range(NCORES):
        s = slice(c * BC, (c + 1) * BC)
        m = dict(consts)
        m["zxT"] = np.ascontiguousarray(zxT[:, s])
        m["lprT"] = np.ascontiguousarray(lprT[:, s])
        in_maps.append(m)
    return in_maps


def run(inputs, trace=False):
    from concourse.bass_utils import run_bass_kernel_spmd

    if "nc" not in _cache:
        _cache["nc"] = _build_nc()
    in_maps = _prep(inputs)
    res = run_bass_kernel_spmd(
        _cache["nc"], in_maps, core_ids=list(range(NCORES)), trace=trace
    )
    # per core: [NW, 128, (blk, t, j)]; batch = w*256 + blk*128 + p
    def decode(a):
        a = a.reshape(NW, 128, 2, T, 2)              # w p blk t j
        return a.transpose(0, 2, 1, 3, 4).reshape(BC, T, 2)
    pred = np.concatenate(
        [decode(r["pred"]) for r in res.results], axis=0
    )  # [B, T, 2]
    out = pred.transpose(1, 0, 2) + np.asarray(inputs["b_hp"], np.float32)[None, None, :]
    return np.ascontiguousarray(out), res


def kernel(**inputs) -> np.ndarray:
    out, _ = run(inputs, trace=False)
    return out



# revision 27
# speedup vs baseline: 1.0257x; 1.0009x over previous
"""Trainium2 Bass kernel for the trajectory-decoder LSTM problem.

Math (mirrors the reference, with algebraic folds):
  dec_inp feeds gates only through W_emb; W_sp/W_emb/W_hp collapse:
    W_es = W_emb @ W_sp            [4H, 2]
    gates_t = zx@W_zx.T + bias + r_{t-1}@W_es.T + h_{t-1}@W_hh.T
  For t>=1, r_{t-1} = h_{t-1}@W_hp.T + b_hp, so with
    W_hh' = W_hh + W_es @ W_hp,  bias1 = b_ih + b_hh + W_emb@b_sp + W_es@b_hp
  every step becomes uniform:  gates_t = zx@W_zx.T + bias1 + h_{t-1}@W_hh'.T
  The step-0 state needs no r_init projection: since the W_es@b_hp term
  of bias1 cancels,  gates_0 = zxp + bias1 + h_init@W_hh.T
                               + (lpr - b_hp)@W_es.T
  and the step-0 update adds h_0@W_hh'.T - h_init@W_hh.T - lprm@W_es.T,
  whose last two terms depend only on consts + h_init and issue right
  after sigmoid_0 (off the elementwise critical path).
  `last_pos` is dead code (output is just the stacked rel_pos).

Device strategy (pure data-parallel over 8 cores, 4096 batch each):
  - batch on the free dim, features on partitions
  - per 256-batch wave, the 4 gate pre-activations live RESIDENT in one
    2-bank PSUM tile; each step the PE accumulates (h_t - h_{t-1}) @ W_hh'.T
    into it (start=False), so no per-step zxp re-add on the vector engine.
    A K=2 "bank-open" matmul (bias x 0/1 indicator) clears has_written for
    the whole bank and injects the per-gate bias.
  - the g-gate weights are doubled on host so ONE sigmoid over all 4 banks
    yields sig(i),sig(f),sig(o),sig(2g); tanh(g)=2*sig(2g)-1 folds into the
    DVE scalar_tensor_tensor ops for m1 and c.
  - per-step elementwise: sigmoid (ACT), then m1/m2/c and h/dh as
    back-to-back DVE ops (same-queue chaining avoids cross-engine
    semaphore hops on the critical path); tanh on ACT.
  - rel_pos: h-stationary matmuls (batch on out partitions, N=2) into
    small scratch PSUM tiles, copied out via ACT+DVE, one contiguous
    partition-major DMA per wave.
  - waves are software-pipelined on a virtual timeline: every op gets
    an estimated execution time tau and ops are emitted sorted by tau,
    so each engine's in-order queue matches true readiness order.
    Chains step every L=5us; waves start every WS=23.4us (PE-
    feasibility bound). Filler matmuls (mlp/init) are chopped into
    <=0.7us chunks; weight DMAs go on the idle GpSimd queue, chunked
    so pair 0's first mlp unit starts after ~0.7MB.
  - all matmul operands fp16 (full PE rate), PSUM accumulation fp32;
    c kept in fp16, b_hp added on host after gather.
"""

import os
import numpy as np

B = 32768
NCORES = 8
BC = B // NCORES          # 4096 batch per core
WAVE = 256                # batch per recurrence chain (2 PSUM banks of gates)
NW = BC // WAVE           # 16 waves
PAIR = 2 * WAVE           # phase-A (mlp) runs at N=512 across wave pairs
T = 12                    # decode steps
H = 128
G4 = 4 * H                # 512 gate features
ZX = 1056
KP = 1152                 # ZX padded to 9*128
KT = KP // 128            # 9 contraction tiles
MLP = 1024
EMB = 64

_cache = {}


def _build_nc():
    import concourse.bass as bass
    import concourse.bacc as bacc
    import concourse.mybir as mybir
    import concourse.tile as tile
    from concourse.bass import ts

    f16 = mybir.dt.float16
    f32 = mybir.dt.float32
    AF = mybir.ActivationFunctionType
    OP = mybir.AluOpType

    nc = bacc.Bacc("TRN2", target_bir_lowering=False)

    zxT = nc.dram_tensor("zxT", [KP, BC], f16, kind="ExternalInput")
    lprT = nc.dram_tensor("lprT", [2, BC], f16, kind="ExternalInput")
    w1t = nc.dram_tensor("w1t", [128, KT, MLP], f16, kind="ExternalInput")
    wzxt = nc.dram_tensor("wzxt", [128, KT, G4], f16, kind="ExternalInput")
    w2t = nc.dram_tensor("w2t", [128, 8, H], f16, kind="ExternalInput")
    whht = nc.dram_tensor("whht", [128, G4], f16, kind="ExternalInput")
    whpt = nc.dram_tensor("whpt", [128, 2], f16, kind="ExternalInput")
    k3 = nc.dram_tensor("k3", [2, G4], f16, kind="ExternalInput")   # -W_es.T
    wes = nc.dram_tensor("wes", [2, G4], f16, kind="ExternalInput")  # +W_es.T
    bias2 = nc.dram_tensor("bias2", [2, 2 * 128], f16, kind="ExternalInput")
    ind = nc.dram_tensor("ind", [2, 2 * WAVE], f16, kind="ExternalInput")
    bias4 = nc.dram_tensor("bias4", [4, 128], f16, kind="ExternalInput")
    ind4 = nc.dram_tensor("ind4", [4, 512], f16, kind="ExternalInput")
    b1 = nc.dram_tensor("b1", [128, 8], f32, kind="ExternalInput")
    b2 = nc.dram_tensor("b2", [128, 1], f32, kind="ExternalInput")
    whh0 = nc.dram_tensor("whh0", [128, G4], f16, kind="ExternalInput")
    whhn = nc.dram_tensor("whhn", [128, G4], f16, kind="ExternalInput")
    # per wave: [partition=batch, (blk, t, j)] — partition-major so the
    # rel output DMA is one clean 192B-per-partition descriptor
    pred = nc.dram_tensor("pred", [NW, 128, 4 * T], f32, kind="ExternalOutput")

    with tile.TileContext(nc) as tc:
        with (
            tc.tile_pool(name="consts", bufs=1) as cpool,
            tc.tile_pool(name="zx", bufs=2) as zxpool,
            tc.tile_pool(name="h1", bufs=2) as h1pool,
            tc.tile_pool(name="hc", bufs=10) as hcpool,
            tc.tile_pool(name="acts", bufs=6) as apool,
            tc.tile_pool(name="outs", bufs=3) as opool,
            tc.tile_pool(name="scrps", bufs=2, space="PSUM") as scrpool,
            tc.tile_pool(name="gateps", bufs=3, space="PSUM") as gatepool,
        ):
            # ---- constants: small ones loaded upfront; the big mlp/zxp
            # weights are DMA'd in chunks as tau-events so pair 0's first
            # mlp unit starts after ~0.7MB instead of ~5MB ----
            w1t_s = cpool.tile([128, KT, MLP], f16)
            wzxt_s = cpool.tile([128, KT, G4], f16)
            w2t_s = cpool.tile([128, 8, H], f16)
            whht_s = cpool.tile([128, G4], f16)
            nc.sync.dma_start(whht_s[:], whht[:])
            whpt_s = cpool.tile([128, 2], f16)
            nc.sync.dma_start(whpt_s[:], whpt[:])
            k3_s = cpool.tile([2, G4], f16)
            nc.sync.dma_start(k3_s[:], k3[:])
            wes_s = cpool.tile([2, G4], f16)
            nc.sync.dma_start(wes_s[:], wes[:])
            bias2_s = cpool.tile([2, 2 * 128], f16)
            nc.sync.dma_start(bias2_s[:], bias2[:])
            ind_s = cpool.tile([2, 2 * WAVE], f16)
            nc.sync.dma_start(ind_s[:], ind[:])
            bias4_s = cpool.tile([4, 128], f16)
            nc.sync.dma_start(bias4_s[:], bias4[:])
            ind4_s = cpool.tile([4, 512], f16)
            nc.sync.dma_start(ind4_s[:], ind4[:])
            b1_s = cpool.tile([128, 8], f32)
            nc.sync.dma_start(b1_s[:], b1[:])
            b2_s = cpool.tile([128, 1], f32)
            nc.sync.dma_start(b2_s[:], b2[:])
            whh0_s = cpool.tile([128, G4], f16)
            nc.sync.dma_start(whh0_s[:], whh0[:])
            whhn_s = cpool.tile([128, G4], f16)
            nc.sync.dma_start(whhn_s[:], whhn[:])
            lpr_s = cpool.tile([2, BC], f16)
            nc.sync.dma_start(lpr_s[:], lprT[:])

            zxT_v = zxT.rearrange("(k p) b -> p k b", p=128)

            # Virtual-timeline schedule: every op gets an estimated
            # execution time tau (ns) and ops are emitted sorted by tau,
            # so each engine's in-order queue matches true readiness
            # order. Chains step every L; waves start every WS (set by
            # PE feasibility: ~22us of PE work per wave). Filler matmul
            # bursts (mlp/init/rel) are chopped into <=0.7us chunks so a
            # chain's gate matmuls are never queued behind a long burst.
            NP = NW // 2
            state = [dict() for _ in range(NW)]
            pair_state = [dict() for _ in range(NP)]
            events = []

            L = 5000
            WS = 23400

            def ev(tau, fn):
                events.append((tau, len(events), fn))

            def mk_w1t_dma(jh):
                def fn():
                    eng = nc.gpsimd
                    eng.dma_start(
                        w1t_s[:, :, ts(jh, 256)], w1t[:, :, ts(jh, 256)]
                    )
                return fn

            def mk_wzxt_dma(half):
                def fn():
                    nc.gpsimd.dma_start(
                        wzxt_s[:, :, ts(half, 256)], wzxt[:, :, ts(half, 256)]
                    )
                return fn

            def mk_w2t_dma():
                def fn():
                    nc.sync.dma_start(w2t_s[:], w2t[:])
                return fn

            def mk_zxw(p):
                def fn():
                    st = pair_state[p]
                    zxw = zxpool.tile([128, KT, PAIR], f16, tag="zxw", name="zxw")
                    nc.gpsimd.dma_start(zxw[:], zxT_v[:, :, ts(p, PAIR)])
                    st["zxw"] = zxw
                    st["h1"] = h1pool.tile([128, 8, PAIR], f16, tag="h1", name="h1")
                return fn

            def mk_mlp1_mm(p, j, k0):
                def fn():
                    st = pair_state[p]
                    if k0 == 0:
                        st["ps"] = scrpool.tile([128, PAIR], f32, tag="scratch", name="ps")
                    for k in range(k0, min(k0 + 3, KT)):
                        nc.tensor.matmul(
                            st["ps"][:], w1t_s[:, k, ts(j, 128)], st["zxw"][:, k, :],
                            start=(k == 0), stop=(k == KT - 1),
                        )
                return fn

            def mk_mlp1_ts(p, j):
                def fn():
                    st = pair_state[p]
                    if j % 2 == 0:
                        nc.scalar.activation(
                            st["h1"][:, j, :], st["ps"][:], AF.Relu,
                            bias=b1_s[:, j : j + 1],
                        )
                    else:
                        nc.vector.tensor_scalar(
                            st["h1"][:, j, :], st["ps"][:], b1_s[:, j : j + 1],
                            0.0, OP.add, OP.max,
                        )
                return fn

            def mk_mlp2_mm(p, j0, nj):
                def fn():
                    st = pair_state[p]
                    if j0 == 0:
                        st["ps"] = scrpool.tile([128, PAIR], f32, tag="scratch", name="ps")
                    for j in range(j0, j0 + nj):
                        nc.tensor.matmul(
                            st["ps"][:], w2t_s[:, j, :], st["h1"][:, j, :],
                            start=(j == 0), stop=(j == 7),
                        )
                return fn

            def mk_mlp2_ts(p):
                def fn():
                    st = pair_state[p]
                    hi = h1pool.tile([128, PAIR], f16, tag="hinit", name="hinit")
                    nc.vector.tensor_scalar(
                        hi[:], st["ps"][:], b2_s[:, 0:1], 0.0, OP.add, OP.max
                    )
                    st["h_init"] = hi
                return fn

            def mk_bias(w):
                def fn():
                    st = state[w]
                    gates = gatepool.tile([128, 4 * WAVE], f32, tag="gates", name="gates")
                    st["gates"] = gates
                    for bk in range(2):
                        nc.tensor.matmul(
                            gates[:, ts(bk, 2 * WAVE)], bias2_s[:, ts(bk, 128)],
                            ind_s[:], start=True, stop=False, skip_group_check=True,
                        )
                return fn

            def mk_zxp(w, g):
                def fn():
                    st = state[w]
                    pst = pair_state[w // 2]
                    hs = ts(w % 2, WAVE)
                    gp = st["gates"][:, ts(g, WAVE)]
                    for k in range(KT):
                        nc.tensor.matmul(
                            gp[:], wzxt_s[:, k, ts(g, 128)], pst["zxw"][:, k, hs],
                            start=False, stop=False, skip_group_check=True,
                        )
                return fn

            # gates_0 = zxp + bias + h_init@W_hh.T + (lpr-b_hp)@W_es.T
            # (the W_es@b_hp term of bias1 cancels; no r_init projection
            # needed, so the pair handoff is just mlp2 -> init_hh -> sig)
            def mk_init_wes(w):
                def fn():
                    st = state[w]
                    for g in range(4):
                        nc.tensor.matmul(
                            st["gates"][:, ts(g, WAVE)], wes_s[:, ts(g, 128)],
                            lpr_s[:, ts(w, WAVE)],
                            start=False, stop=False, skip_group_check=True,
                        )
                return fn

            def mk_init_hh(w):
                def fn():
                    st = state[w]
                    st["h_prev"] = pair_state[w // 2]["h_init"][:, ts(w % 2, WAVE)]
                    for g in range(4):
                        nc.tensor.matmul(
                            st["gates"][:, ts(g, WAVE)], whh0_s[:, ts(g, 128)],
                            st["h_prev"][:],
                            start=False, stop=False, skip_group_check=True,
                        )
                return fn

            # step-0 removal: gates += -h_init@W_hh.T - (lpr-b_hp)@W_es.T;
            # depends only on consts + h_init, issues right after sigmoid_0
            def mk_t0rm(w):
                def fn():
                    st = state[w]
                    for g in range(4):
                        gp = st["gates"][:, ts(g, WAVE)]
                        nc.tensor.matmul(
                            gp[:], whhn_s[:, ts(g, 128)], st["h_prev"][:],
                            start=False, stop=False, skip_group_check=True,
                        )
                        nc.tensor.matmul(
                            gp[:], k3_s[:, ts(g, 128)], lpr_s[:, ts(w, WAVE)],
                            start=False, stop=False, skip_group_check=True,
                        )
                return fn

            def mk_sig(w, t):
                def fn():
                    st = state[w]
                    sig = apool.tile([128, 4 * WAVE], f16, tag="sig", name="sig")
                    nc.scalar.activation(sig[:], st["gates"][:], AF.Sigmoid)
                    st["sig"] = sig
                return fn

            def mk_m1(w, t):
                def fn():
                    st = state[w]
                    m1 = apool.tile([128, WAVE], f16, tag="m1", name="m1")
                    nc.vector.scalar_tensor_tensor(
                        m1[:], st["sig"][:, 3 * WAVE : 4 * WAVE], 0.5,
                        st["sig"][:, 0:WAVE], OP.subtract, OP.mult,
                    )
                    st["m1"] = m1
                return fn

            def mk_m2(w, t):
                # on DVE right after m1: same-queue back-to-back beats the
                # two cross-engine hops of a Pool m2 on the c critical path
                def fn():
                    st = state[w]
                    m2 = apool.tile([128, WAVE], f16, tag="m2", name="m2")
                    nc.vector.tensor_tensor(
                        m2[:], st["sig"][:, WAVE : 2 * WAVE], st["c_prev"][:], OP.mult
                    )
                    st["m2"] = m2
                return fn

            def mk_c(w, t):
                def fn():
                    st = state[w]
                    c_new = hcpool.tile([128, WAVE], f16, tag="c", name="c")
                    if t == 0:
                        nc.vector.tensor_scalar_mul(c_new[:], st["m1"][:], 2.0)
                        st["h_all"] = opool.tile(
                            [128, T * WAVE], f16, tag="hall", name="hall"
                        )
                    else:
                        nc.vector.scalar_tensor_tensor(
                            c_new[:], st["m1"][:], 2.0, st["m2"][:], OP.mult, OP.add
                        )
                    st["c_prev"] = c_new
                return fn

            def mk_tanh(w, t):
                def fn():
                    st = state[w]
                    tanhc = apool.tile([128, WAVE], f16, tag="tanhc", name="tanhc")
                    nc.scalar.activation(tanhc[:], st["c_prev"][:], AF.Tanh)
                    st["tanhc"] = tanhc
                return fn

            def mk_h(w, t):
                def fn():
                    st = state[w]
                    h_new = st["h_all"][:, ts(t, WAVE)]
                    nc.vector.tensor_tensor(
                        h_new[:], st["sig"][:, 2 * WAVE : 3 * WAVE], st["tanhc"][:],
                        OP.mult,
                    )
                    st["h_new"] = h_new
                return fn

            def mk_dh(w, t):
                # DVE back-to-back after h: GpSimd's ~800ns TT plus two
                # cross-engine hops was the fattest slice of step latency
                def fn():
                    st = state[w]
                    dh = apool.tile([128, WAVE], f16, tag="dh", name="dh")
                    nc.vector.tensor_tensor(
                        dh[:], st["h_new"][:], st["h_prev"][:], OP.subtract
                    )
                    st["dh"] = dh
                    st["h_prev"] = st["h_new"]
                return fn

            def mk_mm(w, t):
                def fn():
                    st = state[w]
                    mv = st["h_new"] if t == 0 else st["dh"]
                    for g in range(4):
                        nc.tensor.matmul(
                            st["gates"][:, ts(g, WAVE)], whht_s[:, ts(g, 128)],
                            mv[:],
                            start=False, stop=(t == T - 2), skip_group_check=True,
                        )
                    if t == 0:
                        st["h_prev"] = st["h_new"]
                return fn

            # rel_pos: h-stationary matmuls (batch on out partitions,
            # N=2) into two small scratch PSUM tiles so the gates tile
            # frees at its last sigmoid; split copy across ACT/DVE.
            def mk_rel_mm(w, e0):
                def fn():
                    st = state[w]
                    blk = e0 // T
                    if e0 % T == 0:
                        st["relp%d" % blk] = scrpool.tile(
                            [128, 2 * T], f32, tag="scratch", name="relp"
                        )
                    rp = st["relp%d" % blk]
                    for e in range(e0, e0 + 6):
                        t = e % T
                        nc.tensor.matmul(
                            rp[:, 2 * t : 2 * t + 2],
                            st["h_all"][:, t * WAVE + blk * 128 :
                                         t * WAVE + blk * 128 + 128],
                            whpt_s[:], start=True, stop=True, skip_group_check=True,
                        )
                return fn

            def mk_rel_copy(w):
                # GPSIMD can't read PSUM; split the copy across ACT and DVE
                def fn():
                    st = state[w]
                    sb = opool.tile([128, 4 * T], f32, tag="predsb", name="predsb")
                    nc.scalar.copy(sb[:, 0 : 2 * T], st["relp0"][:])
                    nc.vector.tensor_copy(sb[:, 2 * T : 4 * T], st["relp1"][:])
                    st["predsb"] = sb
                return fn

            def mk_rel_dma(w):
                def fn():
                    st = state[w]
                    nc.sync.dma_start(pred[w], st["predsb"][:])
                return fn

            # --- last wave split into two concurrent 128-wide chains:
            # the tail is latency-bound, and a 128-wide chain's natural
            # step latency is ~2.7us vs ~4.6us at 256 ---
            tst = [dict(), dict()]

            def tk_bias(h):
                def fn():
                    st = tst[h]
                    if h == 0:
                        g = gatepool.tile([128, 4 * WAVE], f32, tag="gates",
                                          name="gates")
                        tst[0]["gt"] = tst[1]["gt"] = g
                    st["gates"] = st["gt"][:, ts(h, 512)]
                    nc.tensor.matmul(
                        st["gates"][:], bias4_s[:], ind4_s[:],
                        start=True, stop=False, skip_group_check=True,
                    )
                return fn

            def tk_zxp(h, g):
                def fn():
                    st = tst[h]
                    pst = pair_state[NP - 1]
                    hs = slice(256 + 128 * h, 384 + 128 * h)
                    for k in range(KT):
                        nc.tensor.matmul(
                            st["gates"][:, ts(g, 128)],
                            wzxt_s[:, k, ts(g, 128)], pst["zxw"][:, k, hs],
                            start=False, stop=False, skip_group_check=True,
                        )
                return fn

            def tk_init(h):
                def fn():
                    st = tst[h]
                    st["h_prev"] = pair_state[NP - 1]["h_init"][
                        :, 256 + 128 * h : 384 + 128 * h]
                    for g in range(4):
                        nc.tensor.matmul(
                            st["gates"][:, ts(g, 128)], wes_s[:, ts(g, 128)],
                            lpr_s[:, BC - 256 + 128 * h : BC - 128 + 128 * h],
                            start=False, stop=False, skip_group_check=True,
                        )
                        nc.tensor.matmul(
                            st["gates"][:, ts(g, 128)], whh0_s[:, ts(g, 128)],
                            st["h_prev"][:],
                            start=False, stop=False, skip_group_check=True,
                        )
                return fn

            def tk_t0rm(h):
                def fn():
                    st = tst[h]
                    for g in range(4):
                        nc.tensor.matmul(
                            st["gates"][:, ts(g, 128)], whhn_s[:, ts(g, 128)],
                            st["h_prev"][:],
                            start=False, stop=False, skip_group_check=True,
                        )
                        nc.tensor.matmul(
                            st["gates"][:, ts(g, 128)], k3_s[:, ts(g, 128)],
                            lpr_s[:, BC - 256 + 128 * h : BC - 128 + 128 * h],
                            start=False, stop=False, skip_group_check=True,
                        )
                return fn

            def tk_sig(h, t):
                def fn():
                    st = tst[h]
                    sig = apool.tile([128, 512], f16, tag="sigS", name="sigS")
                    nc.scalar.activation(sig[:], st["gates"][:], AF.Sigmoid)
                    st["sig"] = sig
                return fn

            def tk_m1(h, t):
                def fn():
                    st = tst[h]
                    m1 = apool.tile([128, 128], f16, tag="m1S", name="m1S")
                    nc.vector.scalar_tensor_tensor(
                        m1[:], st["sig"][:, 384:512], 0.5,
                        st["sig"][:, 0:128], OP.subtract, OP.mult,
                    )
                    st["m1"] = m1
                return fn

            def tk_m2(h, t):
                def fn():
                    st = tst[h]
                    m2 = apool.tile([128, 128], f16, tag="m2S", name="m2S")
                    nc.vector.tensor_tensor(
                        m2[:], st["sig"][:, 128:256], st["c_prev"][:], OP.mult
                    )
                    st["m2"] = m2
                return fn

            def tk_c(h, t):
                def fn():
                    st = tst[h]
                    c_new = hcpool.tile([128, 128], f16, tag="cS", name="cS")
                    if t == 0:
                        nc.vector.tensor_scalar_mul(c_new[:], st["m1"][:], 2.0)
                        st["h_all"] = opool.tile(
                            [128, T * 128], f16, tag="hallS", name="hallS"
                        )
                    else:
                        nc.vector.scalar_tensor_tensor(
                            c_new[:], st["m1"][:], 2.0, st["m2"][:],
                            OP.mult, OP.add,
                        )
                    st["c_prev"] = c_new
                return fn

            def tk_tanh(h, t):
                def fn():
                    st = tst[h]
                    tanhc = apool.tile([128, 128], f16, tag="tanhcS", name="tanhcS")
                    nc.scalar.activation(tanhc[:], st["c_prev"][:], AF.Tanh)
                    st["tanhc"] = tanhc
                return fn

            def tk_h(h, t):
                def fn():
                    st = tst[h]
                    h_new = st["h_all"][:, ts(t, 128)]
                    nc.vector.tensor_tensor(
                        h_new[:], st["sig"][:, 256:384], st["tanhc"][:], OP.mult
                    )
                    st["h_new"] = h_new
                return fn

            def tk_dh(h, t):
                def fn():
                    st = tst[h]
                    dh = apool.tile([128, 128], f16, tag="dhS", name="dhS")
                    nc.vector.tensor_tensor(
                        dh[:], st["h_new"][:], st["h_prev"][:], OP.subtract
                    )
                    st["dh"] = dh
                    st["h_prev"] = st["h_new"]
                return fn

            def tk_mm(h, t):
                def fn():
                    st = tst[h]
                    mv = st["h_new"] if t == 0 else st["dh"]
                    for g in range(4):
                        nc.tensor.matmul(
                            st["gates"][:, ts(g, 128)], whht_s[:, ts(g, 128)],
                            mv[:],
                            start=False, stop=(t == T - 2), skip_group_check=True,
                        )
                    if t == 0:
                        st["h_prev"] = st["h_new"]
                return fn

            def tk_rel_mm(h, e0):
                def fn():
                    st = tst[h]
                    if e0 == 0:
                        st["relp"] = scrpool.tile(
                            [128, 2 * T], f32, tag="scratch", name="relp"
                        )
                    for e in range(e0, e0 + 6):
                        nc.tensor.matmul(
                            st["relp"][:, 2 * e : 2 * e + 2],
                            st["h_all"][:, ts(e, 128)], whpt_s[:],
                            start=True, stop=True, skip_group_check=True,
                        )
                return fn

            def tk_rel_copy(h):
                def fn():
                    st = tst[h]
                    if h == 0:
                        sb = opool.tile([128, 4 * T], f32, tag="predsb",
                                        name="predsb")
                        tst[0]["sb"] = tst[1]["sb"] = sb
                        nc.scalar.copy(sb[:, 0 : 2 * T], st["relp"][:])
                    else:
                        nc.vector.tensor_copy(
                            st["sb"][:, 2 * T : 4 * T], st["relp"][:]
                        )
                return fn

            def tk_rel_dma():
                def fn():
                    nc.sync.dma_start(pred[NW - 1], tst[0]["sb"][:])
                return fn

            for w in range(NW):
                S = w * WS
                if w == NW - 1:
                    LT = 3200
                    for h in range(2):
                        hb = 1600 * h
                        ev(S - 8000 + 100 * h, tk_bias(h))
                        for g in range(4):
                            ev(S - 7800 + 450 * (g + 4 * h), tk_zxp(h, g))
                        ev(S - 3600 + 150 * h, tk_init(h))
                        for t in range(T):
                            tau = S + t * LT + hb
                            ev(tau, tk_sig(h, t))
                            if t == 0:
                                ev(tau + 1730, tk_t0rm(h))
                            ev(tau + 800, tk_m1(h, t))
                            if t > 0:
                                ev(tau + 1090, tk_m2(h, t))
                            ev(tau + 1375, tk_c(h, t))
                            ev(tau + 1630, tk_tanh(h, t))
                            ev(tau + 1980, tk_h(h, t))
                            if t < T - 1:
                                ev(tau + 2205, tk_dh(h, t))
                                ev(tau + 2525, tk_mm(h, t))
                        R = S + 11 * LT + hb
                        ev(R + 2200 + 300 * h, tk_rel_mm(h, 0))
                        ev(R + 2500 + 300 * h, tk_rel_mm(h, 6))
                        ev(R + 3100 + 300 * h, tk_rel_copy(h))
                    ev(S + 11 * LT + 1600 + 3700, tk_rel_dma())
                    continue
                ev(S - 8000, mk_bias(w))
                for g in range(4):
                    ev(S - 7800 + 900 * g, mk_zxp(w, g))
                ev(S - 3600, mk_init_wes(w))
                ev(S - 2600, mk_init_hh(w))
                for t in range(T):
                    tau = S + t * L
                    ev(tau, mk_sig(w, t))
                    if t == 0:
                        ev(tau + 2700, mk_t0rm(w))
                    ev(tau + 1250, mk_m1(w, t))
                    if t > 0:
                        ev(tau + 1700, mk_m2(w, t))
                    ev(tau + 2150, mk_c(w, t))
                    ev(tau + 2550, mk_tanh(w, t))
                    ev(tau + 3100, mk_h(w, t))
                    if t == 0:
                        ev(tau + 3450, mk_mm(w, t))
                    elif t < T - 1:
                        ev(tau + 3450, mk_dh(w, t))
                        ev(tau + 3950, mk_mm(w, t))
                R = S + 11 * L
                for c in range(4):
                    ev(R + 3300 + 600 * c, mk_rel_mm(w, 6 * c))
                ev(R + 5700, mk_rel_copy(w))
                ev(R + 6100, mk_rel_dma(w))
            for p in range(NP):
                E = 2 * p * WS
                ev(-42000 if p == 0 else E - 27800, mk_zxw(p))
                for j in range(8):
                    u = E - 27000 + 2400 * j
                    for ci, k0 in enumerate((0, 3, 6)):
                        ev(u + 750 * ci, mk_mlp1_mm(p, j, k0))
                    ev(u + 2100, mk_mlp1_ts(p, j))
                for ci, (j0, nj) in enumerate(((0, 3), (3, 3), (6, 2))):
                    ev(E - 7600 + 700 * ci, mk_mlp2_mm(p, j0, nj))
                ev(E - 5800, mk_mlp2_ts(p))
            for jh in range(4):
                ev(-40600 + 300 * jh, mk_w1t_dma(jh))
            ev(-39400, mk_wzxt_dma(0))
            ev(-39100, mk_wzxt_dma(1))
            ev(-38800, mk_w2t_dma())

            for _, _, fn in sorted(events, key=lambda e: (e[0], e[1])):
                fn()

    nc.compile()
    return nc


def _prep(inputs):
    """Host-side weight folding + layout prep. Returns per-core input maps."""
    f = np.float64
    W_ih = np.asarray(inputs["W_ih"], f)
    W_hh = np.asarray(inputs["W_hh"], f)
    b_ih = np.asarray(inputs["b_ih"], f)
    b_hh = np.asarray(inputs["b_hh"], f)
    W1 = np.asarray(inputs["W1"], f)
    b1 = np.asarray(inputs["b1"], f)
    W2 = np.asarray(inputs["W2"], f)
    b2 = np.asarray(inputs["b2"], f)
    W_sp = np.asarray(inputs["W_sp"], f)
    b_sp = np.asarray(inputs["b_sp"], f)
    W_hp = np.asarray(inputs["W_hp"], f)
    b_hp = np.asarray(inputs["b_hp"], f)

    W_zx = W_ih[:, :ZX]
    W_emb = W_ih[:, ZX:]
    W_es = W_emb @ W_sp                       # [4H, 2]
    W_hh_f = W_hh + W_es @ W_hp               # [4H, H]
    bias1 = b_ih + b_hh + W_emb @ b_sp + W_es @ b_hp

    # reorder pytorch gates (i, f, g, o) -> (i, f, o, g)
    perm = np.r_[0:H, H : 2 * H, 3 * H : 4 * H, 2 * H : 3 * H]
    W_zx = W_zx[perm]
    W_hh_f = W_hh_f[perm]
    W_es = W_es[perm]
    bias1 = bias1[perm]
    # double the g-gate block: its bank then holds 2*g_pre, so
    # tanh(g) = 2*sigmoid(2*g_pre) - 1 comes out of the one big sigmoid
    dbl = np.ones((G4, 1))
    dbl[3 * H :] = 2.0
    W_zx = W_zx * dbl
    W_hh_f = W_hh_f * dbl
    W_es = W_es * dbl
    bias1 = bias1 * dbl[:, 0]

    def kxm(Wt, kp):  # [K, M] -> [128, K/128, M] fp16, zero-padded to kp rows
        K, M = Wt.shape
        out = np.zeros((kp, M), f)
        out[:K] = Wt
        return np.ascontiguousarray(
            out.reshape(kp // 128, 128, M).transpose(1, 0, 2)
        ).astype(np.float16)

    W_hh_pd = W_hh[perm] * dbl
    consts = {
        "whh0": np.ascontiguousarray(W_hh_pd.T).astype(np.float16),
        "whhn": np.ascontiguousarray(-W_hh_pd.T).astype(np.float16),
        "w1t": kxm(W1.T, KP),
        "wzxt": kxm(W_zx.T, KP),
        "w2t": kxm(W2.T, MLP),
        "whht": np.ascontiguousarray(W_hh_f.T).astype(np.float16),
        "whpt": np.ascontiguousarray(W_hp.T).astype(np.float16),
        "k3": np.ascontiguousarray(-W_es.T).astype(np.float16),
        "wes": np.ascontiguousarray(W_es.T).astype(np.float16),
        # bank-open bias: bias2[r, bk*128+m] = bias1[(2*bk + r)*128 + m]
        "bias2": np.ascontiguousarray(
            bias1.reshape(4, 128).reshape(2, 2, 128).transpose(1, 0, 2).reshape(2, 256)
        ).astype(np.float16),
        # 0/1 indicator selecting which half-bank gets which bias row
        "ind": np.kron(np.eye(2), np.ones((1, WAVE))).astype(np.float16),
        # K=4 open for the 128-wide tail half-waves (4 gate blocks per bank)
        "bias4": np.ascontiguousarray(bias1.reshape(4, 128)).astype(np.float16),
        "ind4": np.kron(np.eye(4), np.ones((1, 128))).astype(np.float16),
        "b1": np.ascontiguousarray(b1.reshape(8, 128).T).astype(np.float32),
        "b2": b2.reshape(128, 1).astype(np.float32),
    }

    enc = np.asarray(inputs["enc_h_feat"], np.float32)
    z = np.asarray(inputs["z"], np.float32)
    lpr = np.asarray(inputs["last_pos_rel"], np.float32)
    zxT = np.zeros((KP, B), np.float16)
    zxT[:MLP] = enc.T
    zxT[MLP:ZX] = z.T
    lprT = np.ascontiguousarray((lpr - b_hp[None, :]).T).astype(np.float16)

    in_maps = []
    for c in range(NCORES):
        s = slice(c * BC, (c + 1) * BC)
        m = dict(consts)
        m["zxT"] = np.ascontiguousarray(zxT[:, s])
        m["lprT"] = np.ascontiguousarray(lprT[:, s])
        in_maps.append(m)
    return in_maps


def run(inputs, trace=False):
    from concourse.bass_utils import run_bass_kernel_spmd

    if "nc" not in _cache:
        _cache["nc"] = _build_nc()
    in_maps = _prep(inputs)
    res = run_bass_kernel_spmd(
        _cache["nc"], in_maps, core_ids=list(range(NCORES)), trace=trace
    )
    # per core: [NW, 128, (blk, t, j)]; batch = w*256 + blk*128 + p
    def decode(a):
        a = a.reshape(NW, 128, 2, T, 2)              # w p blk t j
        return a.transpose(0, 2, 1, 3, 4).reshape(BC, T, 2)
    pred = np.concatenate(
        [decode(r["pred"]) for r in res.results], axis=0
    )  # [B, T, 2]
    out = pred.transpose(1, 0, 2) + np.asarray(inputs["b_hp"], np.float32)[None, None, :]
    return np.ascontiguousarray(out), res


def kernel(**inputs) -> np.ndarray:
    out, _ = run(inputs, trace=False)
    return out



# revision 32
# speedup vs baseline: 1.0347x; 1.0088x over previous
"""Trainium2 Bass kernel for the trajectory-decoder LSTM problem.

Math (mirrors the reference, with algebraic folds):
  dec_inp feeds gates only through W_emb; W_sp/W_emb/W_hp collapse:
    W_es = W_emb @ W_sp            [4H, 2]
    gates_t = zx@W_zx.T + bias + r_{t-1}@W_es.T + h_{t-1}@W_hh.T
  For t>=1, r_{t-1} = h_{t-1}@W_hp.T + b_hp, so with
    W_hh' = W_hh + W_es @ W_hp,  bias1 = b_ih + b_hh + W_emb@b_sp + W_es@b_hp
  every step becomes uniform:  gates_t = zx@W_zx.T + bias1 + h_{t-1}@W_hh'.T
  The step-0 state needs no r_init projection: since the W_es@b_hp term
  of bias1 cancels,  gates_0 = zxp + bias1 + h_init@W_hh.T
                               + (lpr - b_hp)@W_es.T
  and the step-0 update adds h_0@W_hh'.T - h_init@W_hh.T - lprm@W_es.T,
  whose last two terms depend only on consts + h_init and issue right
  after sigmoid_0 (off the elementwise critical path).
  `last_pos` is dead code (output is just the stacked rel_pos).

Device strategy (pure data-parallel over 8 cores, 4096 batch each):
  - batch on the free dim, features on partitions
  - per 256-batch wave, the 4 gate pre-activations live RESIDENT in one
    2-bank PSUM tile; each step the PE accumulates (h_t - h_{t-1}) @ W_hh'.T
    into it (start=False), so no per-step zxp re-add on the vector engine.
    A K=2 "bank-open" matmul (bias x 0/1 indicator) clears has_written for
    the whole bank and injects the per-gate bias.
  - the g-gate weights are doubled on host so ONE sigmoid over all 4 banks
    yields sig(i),sig(f),sig(o),sig(2g); tanh(g)=2*sig(2g)-1 folds into the
    DVE scalar_tensor_tensor ops for m1 and c.
  - per-step elementwise: sigmoid (ACT), then m1/m2/c and h/dh as
    back-to-back DVE ops (same-queue chaining avoids cross-engine
    semaphore hops on the critical path); tanh on ACT.
  - rel_pos: h-stationary matmuls (batch on out partitions, N=2) into
    small scratch PSUM tiles, copied out via ACT+DVE, one contiguous
    partition-major DMA per wave.
  - waves are software-pipelined on a virtual timeline: every op gets
    an estimated execution time tau and ops are emitted sorted by tau,
    so each engine's in-order queue matches true readiness order.
    Chains step every L=5us; waves start every WS=23.4us (PE-
    feasibility bound). Filler matmuls (mlp/init) are chopped into
    <=0.7us chunks; weight DMAs go on the idle GpSimd queue, chunked
    so pair 0's first mlp unit starts after ~0.7MB.
  - all matmul operands fp16 (full PE rate), PSUM accumulation fp32;
    c kept in fp16, b_hp added on host after gather.
"""

import os
import numpy as np

B = 32768
NCORES = 8
BC = B // NCORES          # 4096 batch per core
WAVE = 256                # batch per recurrence chain (2 PSUM banks of gates)
NW = BC // WAVE           # 16 waves
PAIR = 2 * WAVE           # phase-A (mlp) runs at N=512 across wave pairs
T = 12                    # decode steps
H = 128
G4 = 4 * H                # 512 gate features
ZX = 1056
KP = 1152                 # ZX padded to 9*128
KT = KP // 128            # 9 contraction tiles
MLP = 1024
EMB = 64

_cache = {}


def _build_nc():
    import concourse.bass as bass
    import concourse.bacc as bacc
    import concourse.mybir as mybir
    import concourse.tile as tile
    from concourse.bass import ts

    f16 = mybir.dt.float16
    f32 = mybir.dt.float32
    AF = mybir.ActivationFunctionType
    OP = mybir.AluOpType

    nc = bacc.Bacc("TRN2", target_bir_lowering=False)

    zxT = nc.dram_tensor("zxT", [KP, BC], f16, kind="ExternalInput")
    lprT = nc.dram_tensor("lprT", [2, BC], f16, kind="ExternalInput")
    w1t = nc.dram_tensor("w1t", [128, KT, MLP], f16, kind="ExternalInput")
    wzxt = nc.dram_tensor("wzxt", [128, KT, G4], f16, kind="ExternalInput")
    w2t = nc.dram_tensor("w2t", [128, 8, H], f16, kind="ExternalInput")
    whht = nc.dram_tensor("whht", [128, G4], f16, kind="ExternalInput")
    whpt = nc.dram_tensor("whpt", [128, 2], f16, kind="ExternalInput")
    k3 = nc.dram_tensor("k3", [2, G4], f16, kind="ExternalInput")   # -W_es.T
    wes = nc.dram_tensor("wes", [2, G4], f16, kind="ExternalInput")  # +W_es.T
    bias2 = nc.dram_tensor("bias2", [2, 2 * 128], f16, kind="ExternalInput")
    ind = nc.dram_tensor("ind", [2, 2 * WAVE], f16, kind="ExternalInput")
    bias4 = nc.dram_tensor("bias4", [4, 128], f16, kind="ExternalInput")
    ind4 = nc.dram_tensor("ind4", [4, 512], f16, kind="ExternalInput")
    b1 = nc.dram_tensor("b1", [128, 8], f32, kind="ExternalInput")
    b2 = nc.dram_tensor("b2", [128, 1], f32, kind="ExternalInput")
    whh0 = nc.dram_tensor("whh0", [128, G4], f16, kind="ExternalInput")
    whhn = nc.dram_tensor("whhn", [128, G4], f16, kind="ExternalInput")
    # per wave: [partition=batch, (blk, t, j)] — partition-major so the
    # rel output DMA is one clean 192B-per-partition descriptor
    pred = nc.dram_tensor("pred", [NW, 128, 4 * T], f32, kind="ExternalOutput")

    with tile.TileContext(nc) as tc:
        with (
            tc.tile_pool(name="consts", bufs=1) as cpool,
            tc.tile_pool(name="zx", bufs=2) as zxpool,
            tc.tile_pool(name="h1", bufs=2) as h1pool,
            tc.tile_pool(name="hc", bufs=10) as hcpool,
            tc.tile_pool(name="acts", bufs=6) as apool,
            tc.tile_pool(name="outs", bufs=3) as opool,
            tc.tile_pool(name="scrps", bufs=2, space="PSUM") as scrpool,
            tc.tile_pool(name="gateps", bufs=3, space="PSUM") as gatepool,
        ):
            # ---- constants: small ones loaded upfront; the big mlp/zxp
            # weights are DMA'd in chunks as tau-events so pair 0's first
            # mlp unit starts after ~0.7MB instead of ~5MB ----
            w1t_s = cpool.tile([128, KT, MLP], f16)
            wzxt_s = cpool.tile([128, KT, G4], f16)
            w2t_s = cpool.tile([128, 8, H], f16)
            whht_s = cpool.tile([128, G4], f16)
            nc.sync.dma_start(whht_s[:], whht[:])
            whpt_s = cpool.tile([128, 2], f16)
            nc.sync.dma_start(whpt_s[:], whpt[:])
            k3_s = cpool.tile([2, G4], f16)
            nc.sync.dma_start(k3_s[:], k3[:])
            wes_s = cpool.tile([2, G4], f16)
            nc.sync.dma_start(wes_s[:], wes[:])
            bias2_s = cpool.tile([2, 2 * 128], f16)
            nc.sync.dma_start(bias2_s[:], bias2[:])
            ind_s = cpool.tile([2, 2 * WAVE], f16)
            nc.sync.dma_start(ind_s[:], ind[:])
            bias4_s = cpool.tile([4, 128], f16)
            nc.sync.dma_start(bias4_s[:], bias4[:])
            ind4_s = cpool.tile([4, 512], f16)
            nc.sync.dma_start(ind4_s[:], ind4[:])
            b1_s = cpool.tile([128, 8], f32)
            nc.sync.dma_start(b1_s[:], b1[:])
            b2_s = cpool.tile([128, 1], f32)
            nc.sync.dma_start(b2_s[:], b2[:])
            whh0_s = cpool.tile([128, G4], f16)
            nc.sync.dma_start(whh0_s[:], whh0[:])
            whhn_s = cpool.tile([128, G4], f16)
            nc.sync.dma_start(whhn_s[:], whhn[:])
            lpr_s = cpool.tile([2, BC], f16)
            nc.sync.dma_start(lpr_s[:], lprT[:])

            zxT_v = zxT.rearrange("(k p) b -> p k b", p=128)

            # Virtual-timeline schedule: every op gets an estimated
            # execution time tau (ns) and ops are emitted sorted by tau,
            # so each engine's in-order queue matches true readiness
            # order. Chains step every L; waves start every WS (set by
            # PE feasibility: ~22us of PE work per wave). Filler matmul
            # bursts (mlp/init/rel) are chopped into <=0.7us chunks so a
            # chain's gate matmuls are never queued behind a long burst.
            NP = NW // 2
            state = [dict() for _ in range(NW)]
            pair_state = [dict() for _ in range(NP)]
            events = []

            L = 5000
            WS = 23400

            def ev(tau, fn):
                events.append((tau, len(events), fn))

            def mk_w1t_dma(jh):
                def fn():
                    eng = nc.gpsimd
                    eng.dma_start(
                        w1t_s[:, :, ts(jh, 256)], w1t[:, :, ts(jh, 256)]
                    )
                return fn

            def mk_wzxt_dma(half):
                def fn():
                    nc.gpsimd.dma_start(
                        wzxt_s[:, :, ts(half, 256)], wzxt[:, :, ts(half, 256)]
                    )
                return fn

            def mk_w2t_dma():
                def fn():
                    nc.sync.dma_start(w2t_s[:], w2t[:])
                return fn

            def mk_zxw(p):
                def fn():
                    st = pair_state[p]
                    zxw = zxpool.tile([128, KT, PAIR], f16, tag="zxw", name="zxw")
                    nc.gpsimd.dma_start(zxw[:], zxT_v[:, :, ts(p, PAIR)])
                    st["zxw"] = zxw
                    st["h1"] = h1pool.tile([128, 8, PAIR], f16, tag="h1", name="h1")
                return fn

            def mk_mlp1_mm(p, j, k0):
                def fn():
                    st = pair_state[p]
                    if k0 == 0:
                        st["ps"] = scrpool.tile([128, PAIR], f32, tag="scratch", name="ps")
                    for k in range(k0, min(k0 + 3, KT)):
                        nc.tensor.matmul(
                            st["ps"][:], w1t_s[:, k, ts(j, 128)], st["zxw"][:, k, :],
                            start=(k == 0), stop=(k == KT - 1),
                        )
                return fn

            def mk_mlp1_ts(p, j):
                def fn():
                    st = pair_state[p]
                    if j % 2 == 0:
                        nc.scalar.activation(
                            st["h1"][:, j, :], st["ps"][:], AF.Relu,
                            bias=b1_s[:, j : j + 1],
                        )
                    else:
                        nc.vector.tensor_scalar(
                            st["h1"][:, j, :], st["ps"][:], b1_s[:, j : j + 1],
                            0.0, OP.add, OP.max,
                        )
                return fn

            def mk_mlp2_mm(p, j0, nj):
                def fn():
                    st = pair_state[p]
                    if j0 == 0:
                        st["ps"] = scrpool.tile([128, PAIR], f32, tag="scratch", name="ps")
                    for j in range(j0, j0 + nj):
                        nc.tensor.matmul(
                            st["ps"][:], w2t_s[:, j, :], st["h1"][:, j, :],
                            start=(j == 0), stop=(j == 7),
                        )
                return fn

            def mk_mlp2_ts(p):
                def fn():
                    st = pair_state[p]
                    hi = h1pool.tile([128, PAIR], f16, tag="hinit", name="hinit")
                    nc.vector.tensor_scalar(
                        hi[:], st["ps"][:], b2_s[:, 0:1], 0.0, OP.add, OP.max
                    )
                    st["h_init"] = hi
                return fn

            def mk_bias(w):
                def fn():
                    st = state[w]
                    gates = gatepool.tile([128, 4 * WAVE], f32, tag="gates", name="gates")
                    st["gates"] = gates
                    for bk in range(2):
                        nc.tensor.matmul(
                            gates[:, ts(bk, 2 * WAVE)], bias2_s[:, ts(bk, 128)],
                            ind_s[:], start=True, stop=False, skip_group_check=True,
                        )
                return fn

            def mk_zxp(w, g):
                def fn():
                    st = state[w]
                    pst = pair_state[w // 2]
                    hs = ts(w % 2, WAVE)
                    gp = st["gates"][:, ts(g, WAVE)]
                    for k in range(KT):
                        nc.tensor.matmul(
                            gp[:], wzxt_s[:, k, ts(g, 128)], pst["zxw"][:, k, hs],
                            start=False, stop=False, skip_group_check=True,
                        )
                return fn

            # gates_0 = zxp + bias + h_init@W_hh.T + (lpr-b_hp)@W_es.T
            # (the W_es@b_hp term of bias1 cancels; no r_init projection
            # needed, so the pair handoff is just mlp2 -> init_hh -> sig)
            def mk_init_wes(w):
                def fn():
                    st = state[w]
                    for g in range(4):
                        nc.tensor.matmul(
                            st["gates"][:, ts(g, WAVE)], wes_s[:, ts(g, 128)],
                            lpr_s[:, ts(w, WAVE)],
                            start=False, stop=False, skip_group_check=True,
                        )
                return fn

            def mk_init_hh(w):
                def fn():
                    st = state[w]
                    st["h_prev"] = pair_state[w // 2]["h_init"][:, ts(w % 2, WAVE)]
                    for g in range(4):
                        nc.tensor.matmul(
                            st["gates"][:, ts(g, WAVE)], whh0_s[:, ts(g, 128)],
                            st["h_prev"][:],
                            start=False, stop=False, skip_group_check=True,
                        )
                return fn

            # step-0 removal: gates += -h_init@W_hh.T - (lpr-b_hp)@W_es.T;
            # depends only on consts + h_init, issues right after sigmoid_0
            def mk_t0rm(w):
                def fn():
                    st = state[w]
                    for g in range(4):
                        gp = st["gates"][:, ts(g, WAVE)]
                        nc.tensor.matmul(
                            gp[:], whhn_s[:, ts(g, 128)], st["h_prev"][:],
                            start=False, stop=False, skip_group_check=True,
                        )
                        nc.tensor.matmul(
                            gp[:], k3_s[:, ts(g, 128)], lpr_s[:, ts(w, WAVE)],
                            start=False, stop=False, skip_group_check=True,
                        )
                return fn

            def mk_sig(w, t):
                def fn():
                    st = state[w]
                    sig = apool.tile([128, 4 * WAVE], f16, tag="sig", name="sig")
                    nc.scalar.activation(sig[:], st["gates"][:], AF.Sigmoid)
                    st["sig"] = sig
                return fn

            def mk_m1(w, t):
                def fn():
                    st = state[w]
                    m1 = apool.tile([128, WAVE], f16, tag="m1", name="m1")
                    nc.vector.scalar_tensor_tensor(
                        m1[:], st["sig"][:, 3 * WAVE : 4 * WAVE], 0.5,
                        st["sig"][:, 0:WAVE], OP.subtract, OP.mult,
                    )
                    st["m1"] = m1
                return fn

            def mk_m2(w, t):
                # on DVE right after m1: same-queue back-to-back beats the
                # two cross-engine hops of a Pool m2 on the c critical path
                def fn():
                    st = state[w]
                    m2 = apool.tile([128, WAVE], f16, tag="m2", name="m2")
                    nc.vector.tensor_tensor(
                        m2[:], st["sig"][:, WAVE : 2 * WAVE], st["c_prev"][:], OP.mult
                    )
                    st["m2"] = m2
                return fn

            def mk_c(w, t):
                def fn():
                    st = state[w]
                    c_new = hcpool.tile([128, WAVE], f16, tag="c", name="c")
                    if t == 0:
                        nc.vector.tensor_scalar_mul(c_new[:], st["m1"][:], 2.0)
                        st["h_all"] = opool.tile(
                            [128, T * WAVE], f16, tag="hall", name="hall"
                        )
                    else:
                        nc.vector.scalar_tensor_tensor(
                            c_new[:], st["m1"][:], 2.0, st["m2"][:], OP.mult, OP.add
                        )
                    st["c_prev"] = c_new
                return fn

            def mk_tanh(w, t):
                def fn():
                    st = state[w]
                    tanhc = apool.tile([128, WAVE], f16, tag="tanhc", name="tanhc")
                    nc.scalar.activation(tanhc[:], st["c_prev"][:], AF.Tanh)
                    st["tanhc"] = tanhc
                return fn

            def mk_h(w, t):
                def fn():
                    st = state[w]
                    h_new = st["h_all"][:, ts(t, WAVE)]
                    nc.vector.tensor_tensor(
                        h_new[:], st["sig"][:, 2 * WAVE : 3 * WAVE], st["tanhc"][:],
                        OP.mult,
                    )
                    st["h_new"] = h_new
                return fn

            def mk_dh(w, t):
                # DVE back-to-back after h: GpSimd's ~800ns TT plus two
                # cross-engine hops was the fattest slice of step latency
                def fn():
                    st = state[w]
                    dh = apool.tile([128, WAVE], f16, tag="dh", name="dh")
                    nc.vector.tensor_tensor(
                        dh[:], st["h_new"][:], st["h_prev"][:], OP.subtract
                    )
                    st["dh"] = dh
                    st["h_prev"] = st["h_new"]
                return fn

            def mk_mm(w, t):
                def fn():
                    st = state[w]
                    mv = st["h_new"] if t == 0 else st["dh"]
                    for g in range(4):
                        nc.tensor.matmul(
                            st["gates"][:, ts(g, WAVE)], whht_s[:, ts(g, 128)],
                            mv[:],
                            start=False, stop=(t == T - 2), skip_group_check=True,
                        )
                    if t == 0:
                        st["h_prev"] = st["h_new"]
                return fn

            # rel_pos: h-stationary matmuls (batch on out partitions,
            # N=2) into two small scratch PSUM tiles so the gates tile
            # frees at its last sigmoid; split copy across ACT/DVE.
            def mk_rel_mm(w, e0):
                def fn():
                    st = state[w]
                    blk = e0 // T
                    if e0 % T == 0:
                        st["relp%d" % blk] = scrpool.tile(
                            [128, 2 * T], f32, tag="scratch", name="relp"
                        )
                    rp = st["relp%d" % blk]
                    for e in range(e0, e0 + 6):
                        t = e % T
                        nc.tensor.matmul(
                            rp[:, 2 * t : 2 * t + 2],
                            st["h_all"][:, t * WAVE + blk * 128 :
                                         t * WAVE + blk * 128 + 128],
                            whpt_s[:], start=True, stop=True, skip_group_check=True,
                        )
                return fn

            def mk_rel_copy(w):
                # GPSIMD can't read PSUM; split the copy across ACT and DVE
                def fn():
                    st = state[w]
                    sb = opool.tile([128, 4 * T], f32, tag="predsb", name="predsb")
                    nc.scalar.copy(sb[:, 0 : 2 * T], st["relp0"][:])
                    nc.vector.tensor_copy(sb[:, 2 * T : 4 * T], st["relp1"][:])
                    st["predsb"] = sb
                return fn

            def mk_rel_dma(w):
                def fn():
                    st = state[w]
                    nc.sync.dma_start(pred[w], st["predsb"][:])
                return fn

            # --- last wave split into two concurrent 128-wide chains:
            # the tail is latency-bound, and a 128-wide chain's natural
            # step latency is ~2.7us vs ~4.6us at 256 ---
            tst = [dict(), dict()]

            def tk_bias(h):
                def fn():
                    st = tst[h]
                    if h == 0:
                        g = gatepool.tile([128, 4 * WAVE], f32, tag="gates",
                                          name="gates")
                        tst[0]["gt"] = tst[1]["gt"] = g
                    st["gates"] = st["gt"][:, ts(h, 512)]
                    nc.tensor.matmul(
                        st["gates"][:], bias4_s[:], ind4_s[:],
                        start=True, stop=False, skip_group_check=True,
                    )
                return fn

            def tk_zxp(h, g):
                def fn():
                    st = tst[h]
                    pst = pair_state[NP - 1]
                    hs = slice(256 + 128 * h, 384 + 128 * h)
                    for k in range(KT):
                        nc.tensor.matmul(
                            st["gates"][:, ts(g, 128)],
                            wzxt_s[:, k, ts(g, 128)], pst["zxw"][:, k, hs],
                            start=False, stop=False, skip_group_check=True,
                        )
                return fn

            def tk_init(h):
                def fn():
                    st = tst[h]
                    st["h_prev"] = pair_state[NP - 1]["h_init"][
                        :, 256 + 128 * h : 384 + 128 * h]
                    for g in range(4):
                        nc.tensor.matmul(
                            st["gates"][:, ts(g, 128)], wes_s[:, ts(g, 128)],
                            lpr_s[:, BC - 256 + 128 * h : BC - 128 + 128 * h],
                            start=False, stop=False, skip_group_check=True,
                        )
                        nc.tensor.matmul(
                            st["gates"][:, ts(g, 128)], whh0_s[:, ts(g, 128)],
                            st["h_prev"][:],
                            start=False, stop=False, skip_group_check=True,
                        )
                return fn

            def tk_t0rm(h):
                def fn():
                    st = tst[h]
                    for g in range(4):
                        nc.tensor.matmul(
                            st["gates"][:, ts(g, 128)], whhn_s[:, ts(g, 128)],
                            st["h_prev"][:],
                            start=False, stop=False, skip_group_check=True,
                        )
                        nc.tensor.matmul(
                            st["gates"][:, ts(g, 128)], k3_s[:, ts(g, 128)],
                            lpr_s[:, BC - 256 + 128 * h : BC - 128 + 128 * h],
                            start=False, stop=False, skip_group_check=True,
                        )
                return fn

            def tk_sig(h, t):
                def fn():
                    st = tst[h]
                    sig = apool.tile([128, 512], f16, tag="sigS", name="sigS")
                    nc.scalar.activation(sig[:], st["gates"][:], AF.Sigmoid)
                    st["sig"] = sig
                return fn

            def tk_m1(h, t):
                def fn():
                    st = tst[h]
                    m1 = apool.tile([128, 128], f16, tag="m1S", name="m1S")
                    nc.vector.scalar_tensor_tensor(
                        m1[:], st["sig"][:, 384:512], 0.5,
                        st["sig"][:, 0:128], OP.subtract, OP.mult,
                    )
                    st["m1"] = m1
                return fn

            def tk_m2(h, t):
                def fn():
                    st = tst[h]
                    m2 = apool.tile([128, 128], f16, tag="m2S", name="m2S")
                    nc.vector.tensor_tensor(
                        m2[:], st["sig"][:, 128:256], st["c_prev"][:], OP.mult
                    )
                    st["m2"] = m2
                return fn

            def tk_c(h, t):
                def fn():
                    st = tst[h]
                    c_new = hcpool.tile([128, 128], f16, tag="cS", name="cS")
                    if t == 0:
                        nc.vector.tensor_scalar_mul(c_new[:], st["m1"][:], 2.0)
                        st["h_all"] = opool.tile(
                            [128, T * 128], f16, tag="hallS", name="hallS"
                        )
                    else:
                        nc.vector.scalar_tensor_tensor(
                            c_new[:], st["m1"][:], 2.0, st["m2"][:],
                            OP.mult, OP.add,
                        )
                    st["c_prev"] = c_new
                return fn

            def tk_tanh(h, t):
                def fn():
                    st = tst[h]
                    tanhc = apool.tile([128, 128], f16, tag="tanhcS", name="tanhcS")
                    nc.scalar.activation(tanhc[:], st["c_prev"][:], AF.Tanh)
                    st["tanhc"] = tanhc
                return fn

            def tk_h(h, t):
                def fn():
                    st = tst[h]
                    h_new = st["h_all"][:, ts(t, 128)]
                    nc.vector.tensor_tensor(
                        h_new[:], st["sig"][:, 256:384], st["tanhc"][:], OP.mult
                    )
                    st["h_new"] = h_new
                return fn

            def tk_dh(h, t):
                def fn():
                    st = tst[h]
                    dh = apool.tile([128, 128], f16, tag="dhS", name="dhS")
                    nc.vector.tensor_tensor(
                        dh[:], st["h_new"][:], st["h_prev"][:], OP.subtract
                    )
                    st["dh"] = dh
                    st["h_prev"] = st["h_new"]
                return fn

            def tk_mm(h, t):
                def fn():
                    st = tst[h]
                    mv = st["h_new"] if t == 0 else st["dh"]
                    for g in range(4):
                        nc.tensor.matmul(
                            st["gates"][:, ts(g, 128)], whht_s[:, ts(g, 128)],
                            mv[:],
                            start=False, stop=(t == T - 2), skip_group_check=True,
                        )
                    if t == 0:
                        st["h_prev"] = st["h_new"]
                return fn

            def tk_rel_mm(h, e0):
                def fn():
                    st = tst[h]
                    if e0 == 0:
                        st["relp"] = scrpool.tile(
                            [128, 2 * T], f32, tag="scratch", name="relp"
                        )
                    for e in range(e0, e0 + 6):
                        nc.tensor.matmul(
                            st["relp"][:, 2 * e : 2 * e + 2],
                            st["h_all"][:, ts(e, 128)], whpt_s[:],
                            start=True, stop=True, skip_group_check=True,
                        )
                return fn

            def tk_rel_copy(h):
                def fn():
                    st = tst[h]
                    if h == 0:
                        sb = opool.tile([128, 4 * T], f32, tag="predsb",
                                        name="predsb")
                        tst[0]["sb"] = tst[1]["sb"] = sb
                        nc.scalar.copy(sb[:, 0 : 2 * T], st["relp"][:])
                    else:
                        nc.vector.tensor_copy(
                            st["sb"][:, 2 * T : 4 * T], st["relp"][:]
                        )
                return fn

            def tk_rel_dma():
                def fn():
                    nc.sync.dma_start(pred[NW - 1], tst[0]["sb"][:])
                return fn

            for w in range(NW):
                S = w * WS
                if w == NW - 1:
                    LT = 3200
                    for h in range(2):
                        hb = 1600 * h
                        ev(S - 8000 + 100 * h, tk_bias(h))
                        for g in range(4):
                            ev(S - 7800 + 450 * (g + 4 * h), tk_zxp(h, g))
                        ev(S - 3600 + 150 * h, tk_init(h))
                        for t in range(T):
                            tau = S + t * LT + hb
                            ev(tau, tk_sig(h, t))
                            if t == 0:
                                ev(tau + 1730, tk_t0rm(h))
                            ev(tau + 800, tk_m1(h, t))
                            if t > 0:
                                ev(tau + 1090, tk_m2(h, t))
                            ev(tau + 1375, tk_c(h, t))
                            ev(tau + 1630, tk_tanh(h, t))
                            ev(tau + 1980, tk_h(h, t))
                            if t < T - 1:
                                ev(tau + 2205, tk_dh(h, t))
                                ev(tau + 2525, tk_mm(h, t))
                        R = S + 11 * LT + hb
                        ev(R + 2200 + 300 * h, tk_rel_mm(h, 0))
                        ev(R + 2500 + 300 * h, tk_rel_mm(h, 6))
                        ev(R + 3100 + 300 * h, tk_rel_copy(h))
                    ev(S + 11 * LT + 1600 + 3700, tk_rel_dma())
                    continue
                ev(S - 8000, mk_bias(w))
                for g in range(4):
                    ev(S - 7800 + 900 * g, mk_zxp(w, g))
                ev(S - 3600, mk_init_wes(w))
                ev(S - 2600, mk_init_hh(w))
                for t in range(T):
                    tau = S + t * L
                    ev(tau, mk_sig(w, t))
                    if t == 0:
                        ev(tau + 2700, mk_t0rm(w))
                    ev(tau + 1250, mk_m1(w, t))
                    if t > 0:
                        ev(tau + 1700, mk_m2(w, t))
                    ev(tau + 2150, mk_c(w, t))
                    ev(tau + 2550, mk_tanh(w, t))
                    ev(tau + 3100, mk_h(w, t))
                    if t == 0:
                        ev(tau + 3450, mk_mm(w, t))
                    elif t < T - 1:
                        ev(tau + 3450, mk_dh(w, t))
                        ev(tau + 3950, mk_mm(w, t))
                R = S + 11 * L
                for c in range(4):
                    ev(R + 3300 + 600 * c, mk_rel_mm(w, 6 * c))
                ev(R + 5700, mk_rel_copy(w))
                ev(R + 6100, mk_rel_dma(w))
            for p in range(NP):
                E = 2 * p * WS
                ev(-42000 if p == 0 else E - 27800, mk_zxw(p))
                for j in range(8):
                    u = E - 27000 + 2400 * j
                    for ci, k0 in enumerate((0, 3, 6)):
                        ev(u + 750 * ci, mk_mlp1_mm(p, j, k0))
                    ev(u + 2100, mk_mlp1_ts(p, j))
                for ci, (j0, nj) in enumerate(((0, 3), (3, 3), (6, 2))):
                    ev(E - 7600 + 700 * ci, mk_mlp2_mm(p, j0, nj))
                ev(E - 5800, mk_mlp2_ts(p))
            for jh in range(4):
                ev(-40600 + 300 * jh, mk_w1t_dma(jh))
            ev(-39400, mk_wzxt_dma(0))
            ev(-39100, mk_wzxt_dma(1))
            ev(-38800, mk_w2t_dma())

            for _, _, fn in sorted(events, key=lambda e: (e[0], e[1])):
                fn()

    nc.compile()
    return nc


def _prep(inputs):
    """Host-side weight folding + layout prep. Returns per-core input maps."""
    f = np.float64
    W_ih = np.asarray(inputs["W_ih"], f)
    W_hh = np.asarray(inputs["W_hh"], f)
    b_ih = np.asarray(inputs["b_ih"], f)
    b_hh = np.asarray(inputs["b_hh"], f)
    W1 = np.asarray(inputs["W1"], f)
    b1 = np.asarray(inputs["b1"], f)
    W2 = np.asarray(inputs["W2"], f)
    b2 = np.asarray(inputs["b2"], f)
    W_sp = np.asarray(inputs["W_sp"], f)
    b_sp = np.asarray(inputs["b_sp"], f)
    W_hp = np.asarray(inputs["W_hp"], f)
    b_hp = np.asarray(inputs["b_hp"], f)

    W_zx = W_ih[:, :ZX]
    W_emb = W_ih[:, ZX:]
    W_es = W_emb @ W_sp                       # [4H, 2]
    W_hh_f = W_hh + W_es @ W_hp               # [4H, H]
    bias1 = b_ih + b_hh + W_emb @ b_sp + W_es @ b_hp

    # reorder pytorch gates (i, f, g, o) -> (i, f, o, g)
    perm = np.r_[0:H, H : 2 * H, 3 * H : 4 * H, 2 * H : 3 * H]
    W_zx = W_zx[perm]
    W_hh_f = W_hh_f[perm]
    W_es = W_es[perm]
    bias1 = bias1[perm]
    # double the g-gate block: its bank then holds 2*g_pre, so
    # tanh(g) = 2*sigmoid(2*g_pre) - 1 comes out of the one big sigmoid
    dbl = np.ones((G4, 1))
    dbl[3 * H :] = 2.0
    W_zx = W_zx * dbl
    W_hh_f = W_hh_f * dbl
    W_es = W_es * dbl
    bias1 = bias1 * dbl[:, 0]

    def kxm(Wt, kp):  # [K, M] -> [128, K/128, M] fp16, zero-padded to kp rows
        K, M = Wt.shape
        out = np.zeros((kp, M), f)
        out[:K] = Wt
        return np.ascontiguousarray(
            out.reshape(kp // 128, 128, M).transpose(1, 0, 2)
        ).astype(np.float16)

    W_hh_pd = W_hh[perm] * dbl
    consts = {
        "whh0": np.ascontiguousarray(W_hh_pd.T).astype(np.float16),
        "whhn": np.ascontiguousarray(-W_hh_pd.T).astype(np.float16),
        "w1t": kxm(W1.T, KP),
        "wzxt": kxm(W_zx.T, KP),
        "w2t": kxm(W2.T, MLP),
        "whht": np.ascontiguousarray(W_hh_f.T).astype(np.float16),
        "whpt": np.ascontiguousarray(W_hp.T).astype(np.float16),
        "k3": np.ascontiguousarray(-W_es.T).astype(np.float16),
        "wes": np.ascontiguousarray(W_es.T).astype(np.float16),
        # bank-open bias: bias2[r, bk*128+m] = bias1[(2*bk + r)*128 + m]
        "bias2": np.ascontiguousarray(
            bias1.reshape(4, 128).reshape(2, 2, 128).transpose(1, 0, 2).reshape(2, 256)
        ).astype(np.float16),
        # 0/1 indicator selecting which half-bank gets which bias row
        "ind": np.kron(np.eye(2), np.ones((1, WAVE))).astype(np.float16),
        # K=4 open for the 128-wide tail half-waves (4 gate blocks per bank)
        "bias4": np.ascontiguousarray(bias1.reshape(4, 128)).astype(np.float16),
        "ind4": np.kron(np.eye(4), np.ones((1, 128))).astype(np.float16),
        "b1": np.ascontiguousarray(b1.reshape(8, 128).T).astype(np.float32),
        "b2": b2.reshape(128, 1).astype(np.float32),
    }

    enc = np.asarray(inputs["enc_h_feat"], np.float32)
    z = np.asarray(inputs["z"], np.float32)
    lpr = np.asarray(inputs["last_pos_rel"], np.float32)
    zxT = np.zeros((KP, B), np.float16)
    zxT[:MLP] = enc.T
    zxT[MLP:ZX] = z.T
    lprT = np.ascontiguousarray((lpr - b_hp[None, :]).T).astype(np.float16)

    in_maps = []
    for c in range(NCORES):
        s = slice(c * BC, (c + 1) * BC)
        m = dict(consts)
        m["zxT"] = np.ascontiguousarray(zxT[:, s])
        m["lprT"] = np.ascontiguousarray(lprT[:, s])
        in_maps.append(m)
    return in_maps


def run(inputs, trace=False):
    from concourse.bass_utils import run_bass_kernel_spmd

    if "nc" not in _cache:
        _cache["nc"] = _build_nc()
    in_maps = _prep(inputs)
    res = run_bass_kernel_spmd(
        _cache["nc"], in_maps, core_ids=list(range(NCORES)), trace=trace
    )
    # per core: [NW, 128, (blk, t, j)]; batch = w*256 + blk*128 + p
    def decode(a):
        a = a.reshape(NW, 128, 2, T, 2)              # w p blk t j
        return a.transpose(0, 2, 1, 3, 4).reshape(BC, T, 2)
    pred = np.concatenate(
        [decode(r["pred"]) for r in res.results], axis=0
    )  # [B, T, 2]
    out = pred.transpose(1, 0, 2) + np.asarray(inputs["b_hp"], np.float32)[None, None, :]
    return np.ascontiguousarray(out), res


def kernel(**inputs) -> np.ndarray:
    out, _ = run(inputs, trace=False)
    return out

